# revision 5
# baseline (speedup 1.0000x reference)
"""Trainium2 Bass kernel for nn_EnhancedAttention (sparse axial attention +
SE + local-conv gating, fused output scale). v3.

Sharding: pure data-parallel over batch B=32 across 8 cores (4 images/core);
tiny weights replicated.

v3 changes over v2 (495us baseline):
  - x shipped to device as bf16 (host cast), output returned as bf16 and
    upcast on host: halves both DMA directions, kills on-device casts.
  - XC kept h-major; the h->w reorder for the col stage happens inside the
    stage-2 projection matmuls via strided rhs/lhsT access patterns. This
    removes the 74us of strided ACT scatter copies.
  - pw1 packed 4 column-groups into [128,512] PSUM tiles (tile_position
    col tiling): pw1-gelu runs on 128 lanes instead of 16.
  - S matmuls batched 8 pair-chunks per [128,1024] 2-bank PSUM tile; exp
    is one ACT instr per 8 chunks (4/stage vs 16/stage).
  - dw1 w-boundary fixups moved to the idle GPSIMD engine.
  - final multiply emits bf16 (faster DVE mode).
"""

import numpy as np
import ml_dtypes

B, C, H, W = 32, 256, 64, 64
MID = 16
NCORES = 8
IMGS = B // NCORES  # 4
HW = H * W  # 4096
CT = 2  # channel tiles of 128

_cache = {}

BF16 = ml_dtypes.bfloat16


# ----------------------------------------------------------------------------
# Host-side weight preparation
# ----------------------------------------------------------------------------
def host_prep(inp):
    f32 = np.float32
    p = {}
    row_w = np.asarray(inp["row_w"], f32)   # [48, 256]
    row_b = np.asarray(inp["row_b"], f32)
    col_w = np.asarray(inp["col_w"], f32)   # [48, 16]
    col_b = np.asarray(inp["col_b"], f32)
    ax_w = np.asarray(inp["ax_w"], f32)     # [256, 16]
    ax_b = np.asarray(inp["ax_b"], f32)

    # q/k replicated projections: [ct][128, 128]; block r cols 32r:32r+16
    # hold the weight slice, rest zero (SBUF APs must start at 32-aligned
    # partitions, so q and k live in separate tiles)
    qrep = np.zeros((C, 128), f32)
    krep = np.zeros((C, 128), f32)
    for r in range(4):
        qrep[:, 32 * r:32 * r + 16] = row_w[0:16].T
        krep[:, 32 * r:32 * r + 16] = row_w[16:32].T
    p["qrep_wT"] = qrep.reshape(CT, 128, 128).astype(BF16)
    p["krep_wT"] = krep.reshape(CT, 128, 128).astype(BF16)
    qkb = np.zeros((128, 2), f32)
    for r in range(4):
        qkb[32 * r:32 * r + 16, 0] = row_b[0:16]
        qkb[32 * r:32 * r + 16, 1] = row_b[16:32]
    p["qkb_rep"] = qkb
    row_vb = row_b[32:48]
    # v weights [ct][128, 16]
    p["vw"] = row_w[32:48].T.reshape(CT, 128, 16).astype(BF16)

    # col stage (v bias folded)
    q2w32 = np.zeros((16, 32), f32)
    q2w32[:, 0:16] = col_w[0:16].T
    k2w32 = np.zeros((16, 32), f32)
    k2w32[:, 0:16] = col_w[16:32].T
    p["q2w32"] = q2w32.astype(BF16)
    p["k2w32"] = k2w32.astype(BF16)
    q2k2b = np.zeros((128, 2), f32)
    for r in range(4):
        q2k2b[32 * r:32 * r + 16, 0] = col_b[0:16] + col_w[0:16] @ row_vb
        q2k2b[32 * r:32 * r + 16, 1] = col_b[16:32] + col_w[16:32] @ row_vb
    p["q2k2b_rep"] = q2k2b
    p["v2w"] = col_w[32:48].T.astype(BF16)  # [16, 16]
    col_vb = col_b[32:48] + col_w[32:48] @ row_vb

    p["ax_wT"] = ax_w.T.astype(BF16)  # [16, 256]
    axb = ax_b + ax_w @ col_vb
    p["axb_half"] = (0.5 * axb).reshape(CT, 128, 1).astype(f32)

    # conv branch
    dw1 = np.asarray(inp["dw1_w"], f32)[:, 0, 0, :]  # [256, 3]
    dw2 = np.asarray(inp["dw2_w"], f32)[:, 0, :, 0]  # [256, 3]
    dwd = np.zeros((2, 3, CT, 128, 128), f32)
    for ct in range(CT):
        for tap in range(3):
            dwd[0, tap, ct] = np.diag(dw1[128 * ct:128 * (ct + 1), tap])
            dwd[1, tap, ct] = np.diag(dw2[128 * ct:128 * (ct + 1), tap])
    p["dwdiag"] = dwd.astype(BF16)
    # negated dw1 left/right taps for w-boundary corrections (flat-shift fixup)
    dwn = np.zeros((2, CT, 128, 1), f32)
    for ct in range(CT):
        dwn[0, ct, :, 0] = -dw1[128 * ct:128 * (ct + 1), 0]
        dwn[1, ct, :, 0] = -dw1[128 * ct:128 * (ct + 1), 2]
    p["dwd1neg"] = dwn
    p["dwb"] = np.stack([
        np.asarray(inp["dw1_b"], f32).reshape(CT, 128, 1),
        np.asarray(inp["dw2_b"], f32).reshape(CT, 128, 1),
    ])  # [2, CT, 128, 1]
    # pw1: packed 4 column-groups; lhsT [ct][128, 32] (cols 16:32 zero so the
    # matmul writes zeros to the unused partition rows of each 32-group)
    pw1pad = np.zeros((CT, 128, 32), f32)
    pw1w = np.asarray(inp["pw1_w"], f32)[:, :, 0, 0]  # [16, 256]
    for ct in range(CT):
        pw1pad[ct, :, 0:16] = pw1w[:, 128 * ct:128 * (ct + 1)].T
    p["pw1_wT"] = pw1pad.astype(BF16)
    pw1b_rep = np.zeros((128, 1), f32)
    for g in range(4):
        pw1b_rep[32 * g:32 * g + 16, 0] = np.asarray(inp["pw1_b"], f32)
    p["pw1b_rep"] = pw1b_rep
    # selection matrix: out[c] = sum_g lsum128[32g+c]
    sel = np.zeros((128, 16), f32)
    for g in range(4):
        for j in range(16):
            sel[32 * g + j, j] = 1.0
    p["sel128"] = sel
    p["pw2_wT"] = (np.asarray(inp["pw2_w"], f32)[:, :, 0, 0] / HW).T.copy()  # [16, 256] f32
    p["pw2b_half"] = (0.5 * np.asarray(inp["pw2_b"], f32)).reshape(CT, 128, 1).copy()

    # SE (fp32 throughout, tiny)
    p["fc1_wT"] = (np.asarray(inp["fc1_w"], f32) / HW).T.reshape(CT, 128, 16).copy()
    p["fc1b"] = np.asarray(inp["fc1_b"], f32).reshape(16, 1)
    p["fc2_wT"] = np.asarray(inp["fc2_w"], f32).T.copy()  # [16, 256]
    p["fc2b_half"] = (0.5 * np.asarray(inp["fc2_b"], f32)).reshape(CT, 128, 1).copy()

    p["ident"] = np.eye(128, dtype=f32).astype(BF16)

    fwin = np.asarray(inp["fusion_w"], np.float64)
    e = np.exp(fwin - fwin.max())
    fw = e / e.sum()
    p["_K0"] = float(0.5 * (fw[0] + fw[1] + fw[2]) + fw[3])
    p["_s_g"] = float(0.5 * fw[0])
    p["_s_l"] = float(0.5 * fw[1])
    p["_s_ax"] = float(0.5 * fw[2] / HW)
    return p


# ----------------------------------------------------------------------------
# Bass kernel construction
# ----------------------------------------------------------------------------
def build_nc(scalars, n_imgs=IMGS):
    import concourse.bacc as bacc
    import concourse.bass as bass
    import concourse.tile as tile
    from concourse import mybir

    f32 = mybir.dt.float32
    bf16 = mybir.dt.bfloat16
    AX = mybir.AxisListType.X
    OP = mybir.AluOpType
    AF = mybir.ActivationFunctionType

    nc = bacc.Bacc("TRN2", target_bir_lowering=False, debug=False,
                   num_devices=NCORES)

    # ---- DRAM tensors ----
    dx = nc.dram_tensor("x", [n_imgs, C, HW], bf16, kind="ExternalInput")
    dout = nc.dram_tensor("out", [n_imgs, C, HW], bf16, kind="ExternalOutput")
    dw_names = [
        ("dwdiag", [2, 3, CT, 128, 128], bf16), ("dwb", [2, CT, 128, 1], f32),
        ("dwd1neg", [2, CT, 128, 1], f32),
        ("qrep_wT", [CT, 128, 128], bf16), ("krep_wT", [CT, 128, 128], bf16),
        ("qkb_rep", [128, 2], f32),
        ("vw", [CT, 128, 16], bf16),
        ("q2w32", [16, 32], bf16), ("k2w32", [16, 32], bf16),
        ("q2k2b_rep", [128, 2], f32),
        ("v2w", [16, 16], bf16),
        ("ax_wT", [16, 256], bf16), ("axb_half", [CT, 128, 1], f32),
        ("pw1_wT", [CT, 128, 32], bf16), ("pw1b_rep", [128, 1], f32),
        ("sel128", [128, 16], f32),
        ("pw2_wT", [16, 256], f32), ("pw2b_half", [CT, 128, 1], f32),
        ("fc1_wT", [CT, 128, 16], f32), ("fc1b", [16, 1], f32),
        ("fc2_wT", [16, 256], f32), ("fc2b_half", [CT, 128, 1], f32),
        ("ident", [128, 128], bf16),
    ]
    dws = {nm: nc.dram_tensor(nm, sh, dt, kind="ExternalInput")
           for nm, sh, dt in dw_names}

    K0, s_g, s_l, s_ax = (scalars["_K0"], scalars["_s_g"],
                          scalars["_s_l"], scalars["_s_ax"])

    from contextlib import ExitStack
    with tile.TileContext(nc) as tc, ExitStack() as es:
        singles = es.enter_context(tc.tile_pool(name="singles", bufs=1))
        xbp = es.enter_context(tc.tile_pool(name="xbp", bufs=1))
        y1p = es.enter_context(tc.tile_pool(name="y1p", bufs=1))
        scr = es.enter_context(tc.tile_pool(name="scr", bufs=2))
        attp = es.enter_context(tc.tile_pool(name="attp", bufs=2))
        outp = es.enter_context(tc.tile_pool(name="outp", bufs=2))
        tiny = es.enter_context(tc.tile_pool(name="tiny", bufs=8))
        gate = es.enter_context(tc.tile_pool(name="gate", bufs=1))
        # PSUM: big [128,1024] 2-bank x2 = 4; sps [128,512] 1-bank x2 = 2;
        # ops 1-bank x2 = 2  => 8 banks
        ps_big = es.enter_context(tc.tile_pool(name="ps_big", bufs=2, space="PSUM"))
        ps_s = es.enter_context(tc.tile_pool(name="ps_s", bufs=2, space="PSUM"))
        ps_o = es.enter_context(tc.tile_pool(name="ps_o", bufs=2, space="PSUM"))

        # ---- load weights to SBUF (scalar HWDGE queue so the x-input DMAs
        # on the sync queue start immediately) ----
        def wtile(name, shape, dt, src):
            t = singles.tile(shape, dt, tag=name, name=name)
            nc.scalar.dma_start(out=t[:], in_=src)
            return t

        dwd_sb = [[[wtile(f"dwd{st}{tap}{ct}", [128, 128], bf16,
                          dws["dwdiag"][st, tap, ct])
                    for ct in range(CT)] for tap in range(3)] for st in range(2)]
        dwb_sb = [[wtile(f"dwb{st}{ct}", [128, 1], f32, dws["dwb"][st, ct])
                   for ct in range(CT)] for st in range(2)]
        dwn_sb = [[wtile(f"dwn{sd}{ct}", [128, 1], f32, dws["dwd1neg"][sd, ct])
                   for ct in range(CT)] for sd in range(2)]
        qrep_sb = [wtile(f"qrep{ct}", [128, 128], bf16, dws["qrep_wT"][ct])
                   for ct in range(CT)]
        krep_sb = [wtile(f"krep{ct}", [128, 128], bf16, dws["krep_wT"][ct])
                   for ct in range(CT)]
        qkb_sb = wtile("qkb", [128, 2], f32, dws["qkb_rep"][:])
        vw_sb = [wtile(f"vw{ct}", [128, 16], bf16, dws["vw"][ct]) for ct in range(CT)]
        q2w32_sb = wtile("q2w32", [16, 32], bf16, dws["q2w32"][:])
        k2w32_sb = wtile("k2w32", [16, 32], bf16, dws["k2w32"][:])
        q2k2b_sb = wtile("q2k2b", [128, 2], f32, dws["q2k2b_rep"][:])
        v2w_sb = wtile("v2w", [16, 16], bf16, dws["v2w"][:])
        ax_wT_sb = wtile("axwT", [16, 256], bf16, dws["ax_wT"][:])
        axbh_sb = [wtile(f"axbh{ct}", [128, 1], f32, dws["axb_half"][ct])
                   for ct in range(CT)]
        pw1_sb = [wtile(f"pw1{ct}", [128, 32], bf16, dws["pw1_wT"][ct])
                  for ct in range(CT)]
        pw1br_sb = wtile("pw1br", [128, 1], f32, dws["pw1b_rep"][:])
        sel_sb = wtile("sel128", [128, 16], f32, dws["sel128"][:])
        pw2_sb = wtile("pw2", [16, 256], f32, dws["pw2_wT"][:])
        pw2bh_sb = [wtile(f"pw2bh{ct}", [128, 1], f32, dws["pw2b_half"][ct])
                    for ct in range(CT)]
        fc1_sb = [wtile(f"fc1{ct}", [128, 16], f32, dws["fc1_wT"][ct])
                  for ct in range(CT)]
        fc1b_sb = wtile("fc1b", [16, 1], f32, dws["fc1b"][:])
        fc2_sb = wtile("fc2", [16, 256], f32, dws["fc2_wT"][:])
        fc2bh_sb = [wtile(f"fc2bh{ct}", [128, 1], f32, dws["fc2b_half"][ct])
                    for ct in range(CT)]
        ident_sb = wtile("ident", [128, 128], bf16, dws["ident"][:])

        # persistent vt/vt2 ([px, 17] with ones col); ones col written once
        vt = singles.tile([128, 544], bf16, tag="vtP", name="vtP")
        vt3 = vt.rearrange("p (j c) -> p j c", c=17)
        nc.vector.memset(vt3[:, :, 16], 1.0)
        vt2 = singles.tile([128, 544], bf16, tag="vt2P", name="vt2P")
        vt23 = vt2.rearrange("p (j c) -> p j c", c=17)
        nc.vector.memset(vt23[:, :, 16], 1.0)

        # persistent expS tiles [128, 1024] (8 pair-chunks each), fully zeroed
        # once; exp writes the whole tile, then the 64x64 cross blocks are
        # re-zeroed (2 strided DVE memsets) so attn@v runs as one K=128
        # matmul per pair
        expS_t = {}
        for pfx in ("r", "c"):
            for par in range(2):
                e = singles.tile([128, 1024], bf16, tag=f"expS{pfx}{par}",
                                 name=f"expS{pfx}{par}")
                nc.vector.memset(e[:], 0.0)
                expS_t[(pfx, par)] = e

        # persistent bf16 x for all images (DMA'd directly, host pre-cast)
        xb = [[xbp.tile([128, HW], bf16, tag=f"xb{i}_{ct}", name=f"xb{i}_{ct}")
               for ct in range(CT)] for i in range(n_imgs)]
        # gates per image
        tg = [[gate.tile([128, 1], f32, tag=f"tg{i}{ct}", name=f"tg{i}{ct}")
               for ct in range(CT)] for i in range(n_imgs)]
        tl = [[gate.tile([128, 1], f32, tag=f"tl{i}{ct}", name=f"tl{i}{ct}")
               for ct in range(CT)] for i in range(n_imgs)]
        ta_cols = [[gate.tile([128, 4], f32, tag=f"ta{i}{ct}", name=f"ta{i}{ct}")
                    for ct in range(CT)] for i in range(n_imgs)]

        # ==================== Phase A: conv + SE (gelu table) ================
        for i in range(n_imgs):
            for ct in range(CT):
                nc.sync.dma_start(out=xb[i][ct][:],
                                  in_=dx[i, 128 * ct:128 * (ct + 1), :])

            # ---- SE gate ----
            gsum = [tiny.tile([128, 1], f32, tag="gsum", name=f"gsum{i}{ct}")
                    for ct in range(CT)]
            for ct in range(CT):
                nc.vector.reduce_sum(out=gsum[ct][:], in_=xb[i][ct][:], axis=AX)
            fc1ps = ps_o.tile([16, 1], f32, tag="ops", name=f"fc1ps{i}")
            for ct in range(CT):
                nc.tensor.matmul(fc1ps[:], fc1_sb[ct][:], gsum[ct][:],
                                 start=(ct == 0), stop=(ct == 1))
            r1 = tiny.tile([16, 1], f32, tag="r1", name=f"r1{i}")
            nc.scalar.activation(out=r1[:], in_=fc1ps[:], func=AF.Relu,
                                 bias=fc1b_sb[:], scale=1.0)
            for ct in range(CT):
                fc2ps = ps_o.tile([128, 1], f32, tag="ops", name=f"fc2ps{i}{ct}")
                nc.tensor.matmul(fc2ps[:], fc2_sb[:, 128 * ct:128 * (ct + 1)], r1[:])
                nc.scalar.activation(out=tg[i][ct][:], in_=fc2ps[:], func=AF.Tanh,
                                     bias=fc2bh_sb[ct][:], scale=0.5)

            # ---- dw1 (1x3 along w): 2-chunk PSUM pairs ----
            y1 = [y1p.tile([128, HW], bf16, tag=f"y1{ct}", name=f"y1{i}{ct}")
                  for ct in range(CT)]
            for ct in range(CT):
                xb3 = xb[i][ct].rearrange("p (h w) -> p h w", w=64)
                for cp in range(4):  # chunk pairs
                    ps = ps_big.tile([128, 1024], f32, tag="big", name=f"dw1ps{i}{ct}{cp}")
                    ps3 = ps.rearrange("p (h w) -> p h w", w=64)  # 16 h-rows
                    for half in range(2):
                        c = 2 * cp + half
                        o = 512 * c
                        po = 512 * half
                        nc.tensor.matmul(ps[:, po:po + 512], dwd_sb[0][1][ct][:],
                                         xb[i][ct][:, o:o + 512],
                                         start=True, stop=False)
                        lo = 1 if c == 0 else 0
                        nc.tensor.matmul(ps[:, po + lo:po + 512], dwd_sb[0][0][ct][:],
                                         xb[i][ct][:, o + lo - 1:o + 511],
                                         start=False, stop=False)
                        hi = 511 if c == 7 else 512
                        nc.tensor.matmul(ps[:, po:po + hi], dwd_sb[0][2][ct][:],
                                         xb[i][ct][:, o + 1:o + 1 + hi],
                                         start=False, stop=True)
                        # subtract wrapped left tap at w=0 (h>0), right tap at w=63
                        hh = 8 * half
                        lh = 1 if c == 0 else 0
                        nc.vector.scalar_tensor_tensor(
                            out=ps3[:, hh + lh:hh + 8, 0],
                            in0=xb3[:, 8 * c + lh - 1:8 * c + 7, 63],
                            scalar=dwn_sb[0][ct][:], in1=ps3[:, hh + lh:hh + 8, 0],
                            op0=OP.mult, op1=OP.add)
                        rh = 7 if c == 7 else 8
                        nc.vector.scalar_tensor_tensor(
                            out=ps3[:, hh:hh + rh, 63],
                            in0=xb3[:, 8 * c + 1:8 * c + 1 + rh, 0],
                            scalar=dwn_sb[1][ct][:], in1=ps3[:, hh:hh + rh, 63],
                            op0=OP.mult, op1=OP.add)
                    nc.scalar.activation(out=y1[ct][:, 1024 * cp:1024 * (cp + 1)],
                                         in_=ps[:], func=AF.Gelu,
                                         bias=dwb_sb[0][ct][:], scale=1.0)

            # ---- dw2 (3x1 along h) -> gelu -> pw1 (4 col-groups packed) ----
            lsum_cols = tiny.tile([128, 2], f32, tag="lsum_cols", name=f"lsc{i}")
            for cp in range(4):
                y2c = []
                for ct in range(CT):
                    ps = ps_big.tile([128, 1024], f32, tag="big",
                                     name=f"dw2ps{i}{ct}{cp}")
                    for half in range(2):
                        c = 2 * cp + half
                        o = 512 * c
                        po = 512 * half
                        nc.tensor.matmul(ps[:, po:po + 512], dwd_sb[1][1][ct][:],
                                         y1[ct][:, o:o + 512],
                                         start=True, stop=False)
                        if c == 0:
                            nc.tensor.matmul(ps[:, po + 64:po + 512],
                                             dwd_sb[1][0][ct][:],
                                             y1[ct][:, 0:448],
                                             start=False, stop=False)
                        else:
                            nc.tensor.matmul(ps[:, po:po + 512], dwd_sb[1][0][ct][:],
                                             y1[ct][:, o - 64:o + 448],
                                             start=False, stop=False)
                        if c == 7:
                            nc.tensor.matmul(ps[:, po:po + 448], dwd_sb[1][2][ct][:],
                                             y1[ct][:, o + 64:o + 512],
                                             start=False, stop=True)
                        else:
                            nc.tensor.matmul(ps[:, po:po + 512], dwd_sb[1][2][ct][:],
                                             y1[ct][:, o + 64:o + 576],
                                             start=False, stop=True)
                    yc = scr.tile([128, 1024], bf16, tag=f"y2c{ct}",
                                  name=f"y2c{i}{ct}{cp}")
                    nc.scalar.activation(out=yc[:], in_=ps[:], func=AF.Gelu,
                                         bias=dwb_sb[1][ct][:], scale=1.0)
                    y2c.append(yc)
                # pw1: half-chunk c -> col group g=c%4, accumulated over ct.
                # Two [128,512] PSUM tiles per image (cp pairs 0-1 and 2-3).
                if cp % 2 == 0:
                    pw1ps = ps_s.tile([128, 512], f32, tag="sps",
                                      name=f"pw1ps{i}{cp // 2}")
                for half in range(2):
                    g = (2 * cp + half) % 4
                    for ct in range(CT):
                        nc.tensor.matmul(pw1ps[32 * g:32 * g + 32, :],
                                         pw1_sb[ct][:],
                                         y2c[ct][:, 512 * half:512 * half + 512],
                                         start=(ct == 0), stop=(ct == 1),
                                         tile_position=(0, 32 * g))
                if cp % 2 == 1:
                    g3 = scr.tile([128, 512], bf16, tag="g3", name=f"g3{i}{cp // 2}")
                    nc.scalar.activation(out=g3[:], in_=pw1ps[:], func=AF.Gelu,
                                         bias=pw1br_sb[:], scale=1.0,
                                         accum_out=lsum_cols[:, cp // 2:cp // 2 + 1])

            # local gate: fold the 4 col-groups with a tiny matmul, then pw2
            lsum_ps = ps_o.tile([16, 2], f32, tag="ops", name=f"lsps{i}")
            nc.tensor.matmul(lsum_ps[:], sel_sb[:], lsum_cols[:])
            lsum2 = tiny.tile([16, 2], f32, tag="lsum2", name=f"lsum2{i}")
            nc.vector.tensor_copy(out=lsum2[:], in_=lsum_ps[:])
            lsum = tiny.tile([16, 1], f32, tag="lsum", name=f"lsum{i}")
            nc.vector.reduce_sum(out=lsum[:], in_=lsum2[:], axis=AX)
            for ct in range(CT):
                ps = ps_o.tile([128, 1], f32, tag="ops", name=f"pw2ps{i}{ct}")
                nc.tensor.matmul(ps[:], pw2_sb[:, 128 * ct:128 * (ct + 1)], lsum[:])
                nc.scalar.activation(out=tl[i][ct][:], in_=ps[:], func=AF.Tanh,
                                     bias=pw2bh_sb[ct][:], scale=0.5)

        # ==================== Phase B: axial attention (exp table) ===========
        def attention_block(i, qt, kt, vtt, OC_dst, pfx):
            """S^T matmuls (8 pair-chunks per [128,1024] PSUM tile) -> one exp
            -> re-zero cross blocks -> attn@v + denom -> normalize."""
            vt3l = vtt.rearrange("p (j c) -> p j c", c=17)
            for t in range(4):
                Sps = ps_big.tile([128, 1024], f32, tag="big",
                                  name=f"S{pfx}{i}{t}")
                for s in range(8):
                    j = 8 * t + s
                    cch = j // 4
                    r, g = cch % 4, cch // 4
                    sl = slice(32 * r, 32 * r + 16)
                    fo = 512 * g + 128 * (j % 4)
                    nc.tensor.matmul(
                        Sps[:, 128 * s:128 * s + 128],
                        kt[sl, fo:fo + 128], qt[sl, fo:fo + 128],
                        tile_position=(32 * r, 0))
                expS = expS_t[(pfx, t % 2)]
                nc.scalar.activation(out=expS[:], in_=Sps[:], func=AF.Exp,
                                     scale=0.25)
                e4 = expS.rearrange("p (u dh c) -> p u dh c", dh=2, c=64)
                nc.vector.memset(e4[0:64, :, 1], 0.0)
                nc.vector.memset(e4[64:128, :, 0], 0.0)
                Ops = ps_o.tile([128, 136], f32, tag="ops", name=f"O{pfx}{i}{t}")
                for s in range(8):
                    j = 8 * t + s
                    nc.tensor.matmul(
                        Ops[:, 17 * s:17 * s + 17],
                        expS[:, 128 * s:128 * s + 128],
                        vt3l[:, j, :])
                O3 = Ops.rearrange("p (s c) -> p s c", c=17)
                rD = tiny.tile([128, 8], f32, tag="rD", name=f"rD{pfx}{i}{t}")
                nc.vector.reciprocal(out=rD[:], in_=O3[:, :, 16])
                rDb = bass.AP(tensor=rD.tensor, offset=rD.offset,
                              ap=[rD.ap[0], [1, 8], [0, 16]])
                dst3 = OC_dst[:, 128 * t:128 * t + 128].rearrange(
                    "p (s c) -> p s c", c=16)
                nc.vector.tensor_tensor(out=dst3[:], in0=O3[:, :, 0:16],
                                        in1=rDb, op=OP.mult)

        # ---- stage 1: row attention -> XC (h-major), per image ----
        XCs = []
        for i in range(n_imgs):
            # ---- q/k replicated projections: q_sb/k_sb [128, 1024] ----
            # block r rows 32r:32r+16 = chunk 4g+r at cols 512g
            q_sb = attp.tile([128, 1024], bf16, tag="qt", name=f"q{i}")
            k_sb = attp.tile([128, 1024], bf16, tag="kt", name=f"k{i}")
            for g in range(2):
                for rep, dst, bcol in ((qrep_sb, q_sb, 0), (krep_sb, k_sb, 1)):
                    ps = ps_s.tile([128, 512], f32, tag="sps",
                                   name=f"qkps{i}{g}{bcol}")
                    for r in range(4):
                        c = 4 * g + r
                        for ct in range(CT):
                            nc.tensor.matmul(
                                ps[32 * r:32 * r + 32, :],
                                rep[ct][:, 32 * r:32 * r + 32],
                                xb[i][ct][:, 512 * c:512 * c + 512],
                                start=(ct == 0), stop=(ct == 1),
                                tile_position=(0, 32 * r))
                    nc.vector.tensor_scalar(
                        out=dst[:, 512 * g:512 * g + 512], in0=ps[:],
                        scalar1=qkb_sb[:, bcol:bcol + 1], scalar2=None,
                        op0=OP.add)

            # ---- v direct: vt [128, 544] = [px-pair, 17] with ones col ----
            for jb in range(8):
                vps = ps_o.tile([128, 64], f32, tag="ops", name=f"vps{i}{jb}")
                for jj in range(4):
                    j = 4 * jb + jj
                    for ct in range(CT):
                        nc.tensor.matmul(
                            vps[:, 16 * jj:16 * jj + 16],
                            xb[i][ct][:, 128 * j:128 * j + 128],
                            vw_sb[ct][:],
                            start=(ct == 0), stop=(ct == 1))
                vsrc = vps.rearrange("p (j c) -> p j c", c=16)
                nc.vector.tensor_copy(out=vt3[:, 4 * jb:4 * jb + 4, 0:16], in_=vsrc)

            OR = attp.tile([128, 512], bf16, tag="OR", name=f"OR{i}")
            attention_block(i, q_sb, k_sb, vt, OR, "r")

            # ---- transposes + w-major scatter -> XC [16, (w h)] ----
            # flat index = w*64 + 8*b + 2*jj + t; scatter runs on DVE
            XC = attp.tile([16, HW], bf16, tag=f"XCimg{i}", name=f"XC{i}", bufs=1)
            XCv = XC.rearrange("d (w b jj t) -> d b jj t w", w=64, b=8, jj=4, t=2)
            for b in range(8):
                trp = ps_s.tile([16, 512], bf16, tag="sps", name=f"trp{i}{b}")
                for m in range(4):
                    nc.tensor.transpose(trp[:, 128 * m:128 * m + 128],
                                        OR[:, 64 * b + 16 * m:64 * b + 16 * m + 16],
                                        ident_sb[:])
                tsrc = trp.rearrange("d (jj t w) -> d jj t w", jj=4, t=2, w=64)
                nc.vector.tensor_copy(out=XCv[:, b], in_=tsrc)
            XCs.append(XC)

        # ---- stage 2: col attention -> ax -> fusion + output, per image ----
        for i in range(n_imgs):
            XC = XCs[i]
            # ---- col stage: q2/k2 projections ----
            q2_sb = attp.tile([128, 1024], bf16, tag="q2t", name=f"q2{i}")
            k2_sb = attp.tile([128, 1024], bf16, tag="k2t", name=f"k2{i}")
            for g in range(2):
                for w32, dst, bcol in ((q2w32_sb, q2_sb, 0), (k2w32_sb, k2_sb, 1)):
                    ps = ps_s.tile([128, 512], f32, tag="sps",
                                   name=f"qk2ps{i}{g}{bcol}")
                    for r in range(4):
                        c = 4 * g + r
                        nc.tensor.matmul(ps[32 * r:32 * r + 32, :],
                                         w32[:],
                                         XC[:, 512 * c:512 * c + 512],
                                         tile_position=(0, 32 * r))
                    nc.vector.tensor_scalar(
                        out=dst[:, 512 * g:512 * g + 512], in0=ps[:],
                        scalar1=q2k2b_sb[:, bcol:bcol + 1], scalar2=None,
                        op0=OP.add)

            # ---- v2 direct from XC slices ----
            for jb in range(8):
                vps = ps_o.tile([128, 64], f32, tag="ops", name=f"v2ps{i}{jb}")
                for jj in range(4):
                    j = 4 * jb + jj
                    nc.tensor.matmul(
                        vps[:, 16 * jj:16 * jj + 16],
                        XC[:, 128 * j:128 * j + 128],
                        v2w_sb[:])
                vsrc = vps.rearrange("p (j c) -> p j c", c=16)
                nc.vector.tensor_copy(out=vt23[:, 4 * jb:4 * jb + 4, 0:16], in_=vsrc)

            OC = attp.tile([128, 512], bf16, tag="OC", name=f"OC{i}")
            attention_block(i, q2_sb, k2_sb, vt2, OC, "c")

            # ---- transposes -> XC2 (order-free for ax) ----
            XC2 = attp.tile([16, HW], bf16, tag="XC2", name=f"XC2{i}", bufs=1)
            for b in range(8):
                trp = ps_s.tile([16, 512], bf16, tag="sps", name=f"trc{i}{b}")
                for m in range(4):
                    nc.tensor.transpose(trp[:, 128 * m:128 * m + 128],
                                        OC[:, 64 * b + 16 * m:64 * b + 16 * m + 16],
                                        ident_sb[:])
                nc.vector.tensor_copy(out=XC2[:, 512 * b:512 * b + 512], in_=trp[:])

            # ---- ax projection + tanh + accumulated mean ----
            for ct in range(CT):
                for cp in range(4):
                    ps = ps_big.tile([128, 1024], f32, tag="big",
                                     name=f"axps{i}{ct}{cp}")
                    for half in range(2):
                        c = 2 * cp + half
                        nc.tensor.matmul(ps[:, 512 * half:512 * half + 512],
                                         ax_wT_sb[:, 128 * ct:128 * (ct + 1)],
                                         XC2[:, 512 * c:512 * c + 512])
                    axs = scr.tile([128, 1024], bf16, tag="axs", name=f"axs{i}{ct}{cp}")
                    nc.scalar.activation(out=axs[:], in_=ps[:], func=AF.Tanh,
                                         bias=axbh_sb[ct][:], scale=0.5,
                                         accum_out=ta_cols[i][ct][:, cp:cp + 1])

            # ---- fusion + output (bf16, host upcasts) ----
            for ct in range(CT):
                ta = tiny.tile([128, 1], f32, tag="ta", name=f"tafin{i}{ct}")
                nc.vector.reduce_sum(out=ta[:], in_=ta_cols[i][ct][:], axis=AX)
                f0 = tiny.tile([128, 1], f32, tag="f0", name=f"f0{i}{ct}")
                nc.vector.tensor_scalar(out=f0[:], in0=tg[i][ct][:], scalar1=s_g,
                                        scalar2=K0, op0=OP.mult, op1=OP.add)
                f1 = tiny.tile([128, 1], f32, tag="f1", name=f"f1{i}{ct}")
                nc.vector.scalar_tensor_tensor(out=f1[:], in0=tl[i][ct][:],
                                               scalar=s_l, in1=f0[:],
                                               op0=OP.mult, op1=OP.add)
                fin = tiny.tile([128, 1], f32, tag="fin", name=f"fin{i}{ct}")
                nc.vector.scalar_tensor_tensor(out=fin[:], in0=ta[:],
                                               scalar=s_ax, in1=f1[:],
                                               op0=OP.mult, op1=OP.add)
                outt = outp.tile([128, HW], bf16, tag="outt", name=f"outt{i}{ct}")
                nc.vector.tensor_scalar(out=outt[:], in0=xb[i][ct][:],
                                        scalar1=fin[:], scalar2=None,
                                        op0=OP.mult)
                nc.sync.dma_start(out=dout[i, 128 * ct:128 * (ct + 1), :],
                                  in_=outt[:])

    nc.compile()
    return nc


# ----------------------------------------------------------------------------
# Entry point
# ----------------------------------------------------------------------------
WNAMES = ("dwdiag", "dwb", "dwd1neg", "qrep_wT", "krep_wT", "qkb_rep", "vw",
          "q2w32", "k2w32", "q2k2b_rep", "v2w", "ax_wT", "axb_half", "pw1_wT",
          "pw1b_rep", "sel128", "pw2_wT", "pw2b_half", "fc1_wT", "fc1b",
          "fc2_wT", "fc2b_half", "ident")


def kernel(**inputs):
    from concourse.bass_utils import run_bass_kernel_spmd

    p = host_prep(inputs)
    key = "nc"
    if key not in _cache:
        _cache[key] = build_nc(p)
    nc = _cache[key]

    x = np.asarray(inputs["x"], np.float32).reshape(B, C, HW).astype(BF16)
    wmap = {nm: p[nm] for nm in WNAMES}
    in_maps = [{"x": x[IMGS * c:IMGS * (c + 1)], **wmap} for c in range(NCORES)]
    res = run_bass_kernel_spmd(nc, in_maps, list(range(NCORES)))
    _cache["last_results"] = res
    out = np.concatenate([res.results[c]["out"] for c in range(NCORES)], axis=0)
    return out.astype(np.float32).reshape(B, C, H, W)


# revision 13
# speedup vs baseline: 1.1215x; 1.1215x over previous
"""Trainium2 Bass kernel for nn_EnhancedAttention (sparse axial attention +
SE + local-conv gating, fused output scale). v3.

Sharding: pure data-parallel over batch B=32 across 8 cores (4 images/core);
tiny weights replicated.

v4 changes over v2 (495us baseline):
  - x shipped to device as bf16 (host cast), output returned as bf16 and
    upcast on host: halves both DMA directions, kills on-device casts.
  - XC built in two hops: transposes -> contiguous copies into an h-major
    staging tile (DVE), then the h->w scatter runs on the idle GPSIMD
    engine where it hides under compute.
  - stage-1 q/k/v projections emitted inside Phase A so they fill the PE
    bubbles left by the dw-conv gelu dependency chain.
  - attention block software-pipelined: S(t+1) matmuls issue before
    attn@v(t) so the exp/memset latency is off the PE critical path.
  - pw1 packed 4 column-groups into [128,512] PSUM tiles (tile_position
    col tiling): pw1-gelu runs on 128 lanes instead of 16.
  - S matmuls batched 8 pair-chunks per [128,1024] 2-bank PSUM tile; exp
    is one ACT instr per 8 chunks (4/stage vs 16/stage).
  - final multiply emits bf16 (faster DVE mode).
"""

import numpy as np
import ml_dtypes

B, C, H, W = 32, 256, 64, 64
MID = 16
NCORES = 8
IMGS = B // NCORES  # 4
HW = H * W  # 4096
CT = 2  # channel tiles of 128

_cache = {}

BF16 = ml_dtypes.bfloat16


# ----------------------------------------------------------------------------
# Host-side weight preparation
# ----------------------------------------------------------------------------
def host_prep(inp):
    f32 = np.float32
    p = {}
    row_w = np.asarray(inp["row_w"], f32)   # [48, 256]
    row_b = np.asarray(inp["row_b"], f32)
    col_w = np.asarray(inp["col_w"], f32)   # [48, 16]
    col_b = np.asarray(inp["col_b"], f32)
    ax_w = np.asarray(inp["ax_w"], f32)     # [256, 16]
    ax_b = np.asarray(inp["ax_b"], f32)

    # q/k replicated projections: [ct][128, 128]; block r cols 32r:32r+16
    # hold the weight slice, rest zero (SBUF APs must start at 32-aligned
    # partitions, so q and k live in separate tiles)
    qrep = np.zeros((C, 128), f32)
    krep = np.zeros((C, 128), f32)
    for r in range(4):
        qrep[:, 32 * r:32 * r + 16] = row_w[0:16].T
        krep[:, 32 * r:32 * r + 16] = row_w[16:32].T
    p["qrep_wT"] = qrep.reshape(CT, 128, 128).astype(BF16)
    p["krep_wT"] = krep.reshape(CT, 128, 128).astype(BF16)
    qkb = np.zeros((128, 2), f32)
    for r in range(4):
        qkb[32 * r:32 * r + 16, 0] = row_b[0:16]
        qkb[32 * r:32 * r + 16, 1] = row_b[16:32]
    p["qkb_rep"] = qkb
    row_vb = row_b[32:48]
    # v weights [ct][128, 16]
    p["vw"] = row_w[32:48].T.reshape(CT, 128, 16).astype(BF16)

    # col stage (v bias folded)
    q2w32 = np.zeros((16, 32), f32)
    q2w32[:, 0:16] = col_w[0:16].T
    k2w32 = np.zeros((16, 32), f32)
    k2w32[:, 0:16] = col_w[16:32].T
    p["q2w32"] = q2w32.astype(BF16)
    p["k2w32"] = k2w32.astype(BF16)
    q2k2b = np.zeros((128, 2), f32)
    for r in range(4):
        q2k2b[32 * r:32 * r + 16, 0] = col_b[0:16] + col_w[0:16] @ row_vb
        q2k2b[32 * r:32 * r + 16, 1] = col_b[16:32] + col_w[16:32] @ row_vb
    p["q2k2b_rep"] = q2k2b
    p["v2w"] = col_w[32:48].T.astype(BF16)  # [16, 16]
    col_vb = col_b[32:48] + col_w[32:48] @ row_vb

    p["ax_wT"] = ax_w.T.astype(BF16)  # [16, 256]
    axb = ax_b + ax_w @ col_vb
    p["axb_half"] = (0.5 * axb).reshape(CT, 128, 1).astype(f32)

    # conv branch
    dw1 = np.asarray(inp["dw1_w"], f32)[:, 0, 0, :]  # [256, 3]
    dw2 = np.asarray(inp["dw2_w"], f32)[:, 0, :, 0]  # [256, 3]
    dwd = np.zeros((2, 3, CT, 128, 128), f32)
    for ct in range(CT):
        for tap in range(3):
            dwd[0, tap, ct] = np.diag(dw1[128 * ct:128 * (ct + 1), tap])
            dwd[1, tap, ct] = np.diag(dw2[128 * ct:128 * (ct + 1), tap])
    p["dwdiag"] = dwd.astype(BF16)
    # negated dw1 left/right taps for w-boundary corrections (flat-shift fixup)
    dwn = np.zeros((2, CT, 128, 1), f32)
    for ct in range(CT):
        dwn[0, ct, :, 0] = -dw1[128 * ct:128 * (ct + 1), 0]
        dwn[1, ct, :, 0] = -dw1[128 * ct:128 * (ct + 1), 2]
    p["dwd1neg"] = dwn
    p["dwb"] = np.stack([
        np.asarray(inp["dw1_b"], f32).reshape(CT, 128, 1),
        np.asarray(inp["dw2_b"], f32).reshape(CT, 128, 1),
    ])  # [2, CT, 128, 1]
    # pw1: packed 4 column-groups; lhsT [ct][128, 32] (cols 16:32 zero so the
    # matmul writes zeros to the unused partition rows of each 32-group)
    pw1pad = np.zeros((CT, 128, 32), f32)
    pw1w = np.asarray(inp["pw1_w"], f32)[:, :, 0, 0]  # [16, 256]
    for ct in range(CT):
        pw1pad[ct, :, 0:16] = pw1w[:, 128 * ct:128 * (ct + 1)].T
    p["pw1_wT"] = pw1pad.astype(BF16)
    pw1b_rep = np.zeros((128, 1), f32)
    for g in range(4):
        pw1b_rep[32 * g:32 * g + 16, 0] = np.asarray(inp["pw1_b"], f32)
    p["pw1b_rep"] = pw1b_rep
    # selection matrix: out[c] = sum_g lsum128[32g+c]
    sel = np.zeros((128, 16), f32)
    for g in range(4):
        for j in range(16):
            sel[32 * g + j, j] = 1.0
    p["sel128"] = sel
    p["pw2_wT"] = (np.asarray(inp["pw2_w"], f32)[:, :, 0, 0] / HW).T.copy()  # [16, 256] f32
    p["pw2b_half"] = (0.5 * np.asarray(inp["pw2_b"], f32)).reshape(CT, 128, 1).copy()

    # SE (fp32 throughout, tiny)
    p["fc1_wT"] = (np.asarray(inp["fc1_w"], f32) / HW).T.reshape(CT, 128, 16).copy()
    p["fc1b"] = np.asarray(inp["fc1_b"], f32).reshape(16, 1)
    p["fc2_wT"] = np.asarray(inp["fc2_w"], f32).T.copy()  # [16, 256]
    p["fc2b_half"] = (0.5 * np.asarray(inp["fc2_b"], f32)).reshape(CT, 128, 1).copy()

    p["ident"] = np.eye(128, dtype=f32).astype(BF16)

    fwin = np.asarray(inp["fusion_w"], np.float64)
    e = np.exp(fwin - fwin.max())
    fw = e / e.sum()
    p["_K0"] = float(0.5 * (fw[0] + fw[1] + fw[2]) + fw[3])
    p["_s_g"] = float(0.5 * fw[0])
    p["_s_l"] = float(0.5 * fw[1])
    p["_s_ax"] = float(0.5 * fw[2] / HW)
    return p


# ----------------------------------------------------------------------------
# Bass kernel construction
# ----------------------------------------------------------------------------
def build_nc(scalars, n_imgs=IMGS):
    import concourse.bacc as bacc
    import concourse.bass as bass
    import concourse.tile as tile
    from concourse import mybir

    f32 = mybir.dt.float32
    bf16 = mybir.dt.bfloat16
    AX = mybir.AxisListType.X
    OP = mybir.AluOpType
    AF = mybir.ActivationFunctionType

    nc = bacc.Bacc("TRN2", target_bir_lowering=False, debug=False,
                   num_devices=NCORES)

    # ---- DRAM tensors ----
    dx = nc.dram_tensor("x", [n_imgs, C, HW], bf16, kind="ExternalInput")
    dout = nc.dram_tensor("out", [n_imgs, C, HW], bf16, kind="ExternalOutput")
    dw_names = [
        ("dwdiag", [2, 3, CT, 128, 128], bf16), ("dwb", [2, CT, 128, 1], f32),
        ("dwd1neg", [2, CT, 128, 1], f32),
        ("qrep_wT", [CT, 128, 128], bf16), ("krep_wT", [CT, 128, 128], bf16),
        ("qkb_rep", [128, 2], f32),
        ("vw", [CT, 128, 16], bf16),
        ("q2w32", [16, 32], bf16), ("k2w32", [16, 32], bf16),
        ("q2k2b_rep", [128, 2], f32),
        ("v2w", [16, 16], bf16),
        ("ax_wT", [16, 256], bf16), ("axb_half", [CT, 128, 1], f32),
        ("pw1_wT", [CT, 128, 32], bf16), ("pw1b_rep", [128, 1], f32),
        ("sel128", [128, 16], f32),
        ("pw2_wT", [16, 256], f32), ("pw2b_half", [CT, 128, 1], f32),
        ("fc1_wT", [CT, 128, 16], f32), ("fc1b", [16, 1], f32),
        ("fc2_wT", [16, 256], f32), ("fc2b_half", [CT, 128, 1], f32),
        ("ident", [128, 128], bf16),
    ]
    dws = {nm: nc.dram_tensor(nm, sh, dt, kind="ExternalInput")
           for nm, sh, dt in dw_names}

    K0, s_g, s_l, s_ax = (scalars["_K0"], scalars["_s_g"],
                          scalars["_s_l"], scalars["_s_ax"])

    from contextlib import ExitStack
    with tile.TileContext(nc) as tc, ExitStack() as es:
        singles = es.enter_context(tc.tile_pool(name="singles", bufs=1))
        xbp = es.enter_context(tc.tile_pool(name="xbp", bufs=1))
        y1p = es.enter_context(tc.tile_pool(name="y1p", bufs=1))
        scr = es.enter_context(tc.tile_pool(name="scr", bufs=2))
        attp = es.enter_context(tc.tile_pool(name="attp", bufs=2))
        outp = es.enter_context(tc.tile_pool(name="outp", bufs=1))
        tiny = es.enter_context(tc.tile_pool(name="tiny", bufs=8))
        gate = es.enter_context(tc.tile_pool(name="gate", bufs=1))
        # PSUM: big [128,1024] 2-bank x2 = 4; sps [128,512] 1-bank x2 = 2;
        # ops 1-bank x2 = 2  => 8 banks
        ps_big = es.enter_context(tc.tile_pool(name="ps_big", bufs=2, space="PSUM"))
        ps_s = es.enter_context(tc.tile_pool(name="ps_s", bufs=2, space="PSUM"))
        ps_o = es.enter_context(tc.tile_pool(name="ps_o", bufs=2, space="PSUM"))

        # ---- load weights to SBUF (scalar HWDGE queue so the x-input DMAs
        # on the sync queue start immediately) ----
        def wtile(name, shape, dt, src):
            t = singles.tile(shape, dt, tag=name, name=name)
            nc.scalar.dma_start(out=t[:], in_=src)
            return t

        dwd_sb = [[[wtile(f"dwd{st}{tap}{ct}", [128, 128], bf16,
                          dws["dwdiag"][st, tap, ct])
                    for ct in range(CT)] for tap in range(3)] for st in range(2)]
        dwb_sb = [[wtile(f"dwb{st}{ct}", [128, 1], f32, dws["dwb"][st, ct])
                   for ct in range(CT)] for st in range(2)]
        dwn_sb = [[wtile(f"dwn{sd}{ct}", [128, 1], f32, dws["dwd1neg"][sd, ct])
                   for ct in range(CT)] for sd in range(2)]
        qrep_sb = [wtile(f"qrep{ct}", [128, 128], bf16, dws["qrep_wT"][ct])
                   for ct in range(CT)]
        krep_sb = [wtile(f"krep{ct}", [128, 128], bf16, dws["krep_wT"][ct])
                   for ct in range(CT)]
        qkb_sb = wtile("qkb", [128, 2], f32, dws["qkb_rep"][:])
        vw_sb = [wtile(f"vw{ct}", [128, 16], bf16, dws["vw"][ct]) for ct in range(CT)]
        q2w32_sb = wtile("q2w32", [16, 32], bf16, dws["q2w32"][:])
        k2w32_sb = wtile("k2w32", [16, 32], bf16, dws["k2w32"][:])
        q2k2b_sb = wtile("q2k2b", [128, 2], f32, dws["q2k2b_rep"][:])
        v2w_sb = wtile("v2w", [16, 16], bf16, dws["v2w"][:])
        ax_wT_sb = wtile("axwT", [16, 256], bf16, dws["ax_wT"][:])
        axbh_sb = [wtile(f"axbh{ct}", [128, 1], f32, dws["axb_half"][ct])
                   for ct in range(CT)]
        pw1_sb = [wtile(f"pw1{ct}", [128, 32], bf16, dws["pw1_wT"][ct])
                  for ct in range(CT)]
        pw1br_sb = wtile("pw1br", [128, 1], f32, dws["pw1b_rep"][:])
        sel_sb = wtile("sel128", [128, 16], f32, dws["sel128"][:])
        pw2_sb = wtile("pw2", [16, 256], f32, dws["pw2_wT"][:])
        pw2bh_sb = [wtile(f"pw2bh{ct}", [128, 1], f32, dws["pw2b_half"][ct])
                    for ct in range(CT)]
        fc1_sb = [wtile(f"fc1{ct}", [128, 16], f32, dws["fc1_wT"][ct])
                  for ct in range(CT)]
        fc1b_sb = wtile("fc1b", [16, 1], f32, dws["fc1b"][:])
        fc2_sb = wtile("fc2", [16, 256], f32, dws["fc2_wT"][:])
        fc2bh_sb = [wtile(f"fc2bh{ct}", [128, 1], f32, dws["fc2b_half"][ct])
                    for ct in range(CT)]
        ident_sb = wtile("ident", [128, 128], bf16, dws["ident"][:])

        # persistent vt (per image, filled during Phase A) / vt2 ([px, 17]
        # with ones col); ones cols written once
        vt = []
        vt3 = []
        for i in range(n_imgs):
            v = singles.tile([128, 544], bf16, tag=f"vtP{i}", name=f"vtP{i}")
            v3 = v.rearrange("p (j c) -> p j c", c=17)
            nc.vector.memset(v3[:, :, 16], 1.0)
            vt.append(v)
            vt3.append(v3)
        vt2 = singles.tile([128, 544], bf16, tag="vt2P", name="vt2P")
        vt23 = vt2.rearrange("p (j c) -> p j c", c=17)
        nc.vector.memset(vt23[:, :, 16], 1.0)

        # persistent expS tiles [128, 1024] (8 pair-chunks each), fully zeroed
        # once; exp writes the whole tile, then the 64x64 cross blocks are
        # re-zeroed (2 strided DVE memsets) so attn@v runs as one K=128
        # matmul per pair
        expS_t = {}
        for pfx in ("r", "c"):
            for par in range(2):
                e = singles.tile([128, 1024], bf16, tag=f"expS{pfx}{par}",
                                 name=f"expS{pfx}{par}")
                nc.vector.memset(e[:], 0.0)
                expS_t[(pfx, par)] = e

        # persistent bf16 x for all images (DMA'd directly, host pre-cast)
        xb = [[xbp.tile([128, HW], bf16, tag=f"xb{i}_{ct}", name=f"xb{i}_{ct}")
               for ct in range(CT)] for i in range(n_imgs)]
        # gates per image
        tg = [[gate.tile([128, 1], f32, tag=f"tg{i}{ct}", name=f"tg{i}{ct}")
               for ct in range(CT)] for i in range(n_imgs)]
        tl = [[gate.tile([128, 1], f32, tag=f"tl{i}{ct}", name=f"tl{i}{ct}")
               for ct in range(CT)] for i in range(n_imgs)]
        ta_cols = [[gate.tile([128, 4], f32, tag=f"ta{i}{ct}", name=f"ta{i}{ct}")
                    for ct in range(CT)] for i in range(n_imgs)]

        # ==================== Phase A: conv + SE (gelu table) ================
        qk_sbs = []
        for i in range(n_imgs):
            for ct in range(CT):
                nc.sync.dma_start(out=xb[i][ct][:],
                                  in_=dx[i, 128 * ct:128 * (ct + 1), :])

            # ---- SE gate ----
            gsum = [tiny.tile([128, 1], f32, tag="gsum", name=f"gsum{i}{ct}")
                    for ct in range(CT)]
            for ct in range(CT):
                nc.vector.reduce_sum(out=gsum[ct][:], in_=xb[i][ct][:], axis=AX)
            fc1ps = ps_o.tile([16, 1], f32, tag="ops", name=f"fc1ps{i}")
            for ct in range(CT):
                nc.tensor.matmul(fc1ps[:], fc1_sb[ct][:], gsum[ct][:],
                                 start=(ct == 0), stop=(ct == 1))
            r1 = tiny.tile([16, 1], f32, tag="r1", name=f"r1{i}")
            nc.scalar.activation(out=r1[:], in_=fc1ps[:], func=AF.Relu,
                                 bias=fc1b_sb[:], scale=1.0)
            for ct in range(CT):
                fc2ps = ps_o.tile([128, 1], f32, tag="ops", name=f"fc2ps{i}{ct}")
                nc.tensor.matmul(fc2ps[:], fc2_sb[:, 128 * ct:128 * (ct + 1)], r1[:])
                nc.scalar.activation(out=tg[i][ct][:], in_=fc2ps[:], func=AF.Tanh,
                                     bias=fc2bh_sb[ct][:], scale=0.5)

            # ---- dw1 (1x3 along w): 2-chunk PSUM pairs ----
            y1 = [y1p.tile([128, HW], bf16, tag=f"y1{ct}", name=f"y1{i}{ct}")
                  for ct in range(CT)]
            for ct in range(CT):
                xb3 = xb[i][ct].rearrange("p (h w) -> p h w", w=64)
                for cp in range(4):  # chunk pairs
                    ps = ps_big.tile([128, 1024], f32, tag="big", name=f"dw1ps{i}{ct}{cp}")
                    ps3 = ps.rearrange("p (h w) -> p h w", w=64)  # 16 h-rows
                    for half in range(2):
                        c = 2 * cp + half
                        o = 512 * c
                        po = 512 * half
                        nc.tensor.matmul(ps[:, po:po + 512], dwd_sb[0][1][ct][:],
                                         xb[i][ct][:, o:o + 512],
                                         start=True, stop=False)
                        lo = 1 if c == 0 else 0
                        nc.tensor.matmul(ps[:, po + lo:po + 512], dwd_sb[0][0][ct][:],
                                         xb[i][ct][:, o + lo - 1:o + 511],
                                         start=False, stop=False)
                        hi = 511 if c == 7 else 512
                        nc.tensor.matmul(ps[:, po:po + hi], dwd_sb[0][2][ct][:],
                                         xb[i][ct][:, o + 1:o + 1 + hi],
                                         start=False, stop=True)
                        # subtract wrapped left tap at w=0 (h>0), right tap at w=63
                        hh = 8 * half
                        lh = 1 if c == 0 else 0
                        nc.vector.scalar_tensor_tensor(
                            out=ps3[:, hh + lh:hh + 8, 0],
                            in0=xb3[:, 8 * c + lh - 1:8 * c + 7, 63],
                            scalar=dwn_sb[0][ct][:], in1=ps3[:, hh + lh:hh + 8, 0],
                            op0=OP.mult, op1=OP.add)
                        rh = 7 if c == 7 else 8
                        nc.vector.scalar_tensor_tensor(
                            out=ps3[:, hh:hh + rh, 63],
                            in0=xb3[:, 8 * c + 1:8 * c + 1 + rh, 0],
                            scalar=dwn_sb[1][ct][:], in1=ps3[:, hh:hh + rh, 63],
                            op0=OP.mult, op1=OP.add)
                    nc.scalar.activation(out=y1[ct][:, 1024 * cp:1024 * (cp + 1)],
                                         in_=ps[:], func=AF.Gelu,
                                         bias=dwb_sb[0][ct][:], scale=1.0)

            # ---- stage-1 q/k/v projections (emitted here so they fill the
            # PE bubbles left by the dw gelu dependency chain) ----
            # q/k: q_sb/k_sb [128, 1024]; block r rows 32r:32r+16 = chunk
            # 4g+r at cols 512g
            q_sb = attp.tile([128, 1024], bf16, tag="qt", name=f"q{i}", bufs=4)
            k_sb = attp.tile([128, 1024], bf16, tag="kt", name=f"k{i}", bufs=4)
            qk_sbs.append((q_sb, k_sb))
            for g in range(2):
                for rep, dst, bcol in ((qrep_sb, q_sb, 0), (krep_sb, k_sb, 1)):
                    ps = ps_s.tile([128, 512], f32, tag="sps",
                                   name=f"qkps{i}{g}{bcol}")
                    for r in range(4):
                        c = 4 * g + r
                        for ct in range(CT):
                            nc.tensor.matmul(
                                ps[32 * r:32 * r + 32, :],
                                rep[ct][:, 32 * r:32 * r + 32],
                                xb[i][ct][:, 512 * c:512 * c + 512],
                                start=(ct == 0), stop=(ct == 1),
                                tile_position=(0, 32 * r))
                    nc.vector.tensor_scalar(
                        out=dst[:, 512 * g:512 * g + 512], in0=ps[:],
                        scalar1=qkb_sb[:, bcol:bcol + 1], scalar2=None,
                        op0=OP.add)
            # v direct: vt[i] [128, 544] = [px-pair, 17] with ones col
            for jb in range(8):
                vps = ps_o.tile([128, 64], f32, tag="ops", name=f"vps{i}{jb}")
                for jj in range(4):
                    j = 4 * jb + jj
                    for ct in range(CT):
                        nc.tensor.matmul(
                            vps[:, 16 * jj:16 * jj + 16],
                            xb[i][ct][:, 128 * j:128 * j + 128],
                            vw_sb[ct][:],
                            start=(ct == 0), stop=(ct == 1))
                vsrc = vps.rearrange("p (j c) -> p j c", c=16)
                nc.vector.tensor_copy(out=vt3[i][:, 4 * jb:4 * jb + 4, 0:16],
                                      in_=vsrc)

            # ---- dw2 (3x1 along h) -> gelu -> pw1 (4 col-groups packed) ----
            lsum_cols = tiny.tile([128, 2], f32, tag="lsum_cols", name=f"lsc{i}")
            for cp in range(4):
                y2c = []
                for ct in range(CT):
                    ps = ps_big.tile([128, 1024], f32, tag="big",
                                     name=f"dw2ps{i}{ct}{cp}")
                    for half in range(2):
                        c = 2 * cp + half
                        o = 512 * c
                        po = 512 * half
                        nc.tensor.matmul(ps[:, po:po + 512], dwd_sb[1][1][ct][:],
                                         y1[ct][:, o:o + 512],
                                         start=True, stop=False)
                        if c == 0:
                            nc.tensor.matmul(ps[:, po + 64:po + 512],
                                             dwd_sb[1][0][ct][:],
                                             y1[ct][:, 0:448],
                                             start=False, stop=False)
                        else:
                            nc.tensor.matmul(ps[:, po:po + 512], dwd_sb[1][0][ct][:],
                                             y1[ct][:, o - 64:o + 448],
                                             start=False, stop=False)
                        if c == 7:
                            nc.tensor.matmul(ps[:, po:po + 448], dwd_sb[1][2][ct][:],
                                             y1[ct][:, o + 64:o + 512],
                                             start=False, stop=True)
                        else:
                            nc.tensor.matmul(ps[:, po:po + 512], dwd_sb[1][2][ct][:],
                                             y1[ct][:, o + 64:o + 576],
                                             start=False, stop=True)
                    yc = scr.tile([128, 1024], bf16, tag=f"y2c{ct}",
                                  name=f"y2c{i}{ct}{cp}")
                    nc.scalar.activation(out=yc[:], in_=ps[:], func=AF.Gelu,
                                         bias=dwb_sb[1][ct][:], scale=1.0)
                    y2c.append(yc)
                # pw1: half-chunk c -> col group g=c%4, accumulated over ct.
                # Two [128,512] PSUM tiles per image (cp pairs 0-1 and 2-3).
                if cp % 2 == 0:
                    pw1ps = ps_s.tile([128, 512], f32, tag="sps",
                                      name=f"pw1ps{i}{cp // 2}")
                for half in range(2):
                    g = (2 * cp + half) % 4
                    for ct in range(CT):
                        nc.tensor.matmul(pw1ps[32 * g:32 * g + 32, :],
                                         pw1_sb[ct][:],
                                         y2c[ct][:, 512 * half:512 * half + 512],
                                         start=(ct == 0), stop=(ct == 1),
                                         tile_position=(0, 32 * g))
                if cp % 2 == 1:
                    g3 = scr.tile([128, 512], bf16, tag="g3", name=f"g3{i}{cp // 2}")
                    nc.scalar.activation(out=g3[:], in_=pw1ps[:], func=AF.Gelu,
                                         bias=pw1br_sb[:], scale=1.0,
                                         accum_out=lsum_cols[:, cp // 2:cp // 2 + 1])

            # local gate: fold the 4 col-groups with a tiny matmul, then pw2
            lsum_ps = ps_o.tile([16, 2], f32, tag="ops", name=f"lsps{i}")
            nc.tensor.matmul(lsum_ps[:], sel_sb[:], lsum_cols[:])
            lsum2 = tiny.tile([16, 2], f32, tag="lsum2", name=f"lsum2{i}")
            nc.vector.tensor_copy(out=lsum2[:], in_=lsum_ps[:])
            lsum = tiny.tile([16, 1], f32, tag="lsum", name=f"lsum{i}")
            nc.vector.reduce_sum(out=lsum[:], in_=lsum2[:], axis=AX)
            for ct in range(CT):
                ps = ps_o.tile([128, 1], f32, tag="ops", name=f"pw2ps{i}{ct}")
                nc.tensor.matmul(ps[:], pw2_sb[:, 128 * ct:128 * (ct + 1)], lsum[:])
                nc.scalar.activation(out=tl[i][ct][:], in_=ps[:], func=AF.Tanh,
                                     bias=pw2bh_sb[ct][:], scale=0.5)

        # ==================== Phase B: axial attention (exp table) ===========
        def attention_block(i, qt, kt, vtt, OC_dst, pfx):
            """S^T matmuls (8 pair-chunks per [128,1024] PSUM tile) -> one exp
            -> re-zero cross blocks -> attn@v + denom -> normalize.
            Software-pipelined: S(t+1) issues before attn@v(t) so the
            exp/memset chain stays off the PE critical path."""
            vt3l = vtt.rearrange("p (j c) -> p j c", c=17)

            def stage_S(t):
                Sps = ps_big.tile([128, 1024], f32, tag="big",
                                  name=f"S{pfx}{i}{t}")
                for s in range(8):
                    j = 8 * t + s
                    cch = j // 4
                    r, g = cch % 4, cch // 4
                    sl = slice(32 * r, 32 * r + 16)
                    fo = 512 * g + 128 * (j % 4)
                    nc.tensor.matmul(
                        Sps[:, 128 * s:128 * s + 128],
                        kt[sl, fo:fo + 128], qt[sl, fo:fo + 128],
                        tile_position=(32 * r, 0))
                expS = expS_t[(pfx, t % 2)]
                nc.scalar.activation(out=expS[:], in_=Sps[:], func=AF.Exp,
                                     scale=0.25)
                e4 = expS.rearrange("p (u dh c) -> p u dh c", dh=2, c=64)
                nc.vector.memset(e4[0:64, :, 1], 0.0)
                nc.vector.memset(e4[64:128, :, 0], 0.0)

            def stage_AV(t):
                expS = expS_t[(pfx, t % 2)]
                Ops = ps_o.tile([128, 136], f32, tag="ops", name=f"O{pfx}{i}{t}")
                for s in range(8):
                    j = 8 * t + s
                    nc.tensor.matmul(
                        Ops[:, 17 * s:17 * s + 17],
                        expS[:, 128 * s:128 * s + 128],
                        vt3l[:, j, :])
                O3 = Ops.rearrange("p (s c) -> p s c", c=17)
                rD = tiny.tile([128, 8], f32, tag="rD", name=f"rD{pfx}{i}{t}")
                nc.vector.reciprocal(out=rD[:], in_=O3[:, :, 16])
                rDb = bass.AP(tensor=rD.tensor, offset=rD.offset,
                              ap=[rD.ap[0], [1, 8], [0, 16]])
                dst3 = OC_dst[:, 128 * t:128 * t + 128].rearrange(
                    "p (s c) -> p s c", c=16)
                nc.vector.tensor_tensor(out=dst3[:], in0=O3[:, :, 0:16],
                                        in1=rDb, op=OP.mult)

            stage_S(0)
            for t in range(1, 4):
                stage_S(t)
                stage_AV(t - 1)
            stage_AV(3)

        # ---- stage 1: row attention -> XC, per image ----
        XCs = []
        for i in range(n_imgs):
            q_sb, k_sb = qk_sbs[i]
            OR = attp.tile([128, 512], bf16, tag="OR", name=f"OR{i}")
            attention_block(i, q_sb, k_sb, vt[i], OR, "r")

            # ---- transposes -> XCh [16, (h w)] staging (contiguous DVE
            # copies), then h->w scatter to XC [16, (w h)] on idle GPSIMD ----
            XCh = attp.tile([16, HW], bf16, tag="XChst", name=f"XCh{i}", bufs=1)
            for b in range(8):
                trp = ps_s.tile([16, 512], bf16, tag="sps", name=f"trp{i}{b}")
                for m in range(4):
                    nc.tensor.transpose(trp[:, 128 * m:128 * m + 128],
                                        OR[:, 64 * b + 16 * m:64 * b + 16 * m + 16],
                                        ident_sb[:])
                nc.vector.tensor_copy(out=XCh[:, 512 * b:512 * b + 512], in_=trp[:])
            XC = attp.tile([16, HW], bf16, tag=f"XCimg{i}", name=f"XC{i}", bufs=1)
            XCh3 = XCh.rearrange("d (h w) -> d w h", w=64)
            for b in range(8):
                nc.gpsimd.tensor_copy(out=XC[:, 512 * b:512 * b + 512],
                                      in_=XCh3[:, 8 * b:8 * b + 8, :])
            XCs.append(XC)

        # ---- stage 2: col attention -> ax -> fusion + output, per image ----
        for i in range(n_imgs):
            XC = XCs[i]
            # ---- col stage: q2/k2 projections ----
            q2_sb = attp.tile([128, 1024], bf16, tag="q2t", name=f"q2{i}")
            k2_sb = attp.tile([128, 1024], bf16, tag="k2t", name=f"k2{i}")
            for g in range(2):
                for w32, dst, bcol in ((q2w32_sb, q2_sb, 0), (k2w32_sb, k2_sb, 1)):
                    ps = ps_s.tile([128, 512], f32, tag="sps",
                                   name=f"qk2ps{i}{g}{bcol}")
                    for r in range(4):
                        c = 4 * g + r
                        nc.tensor.matmul(ps[32 * r:32 * r + 32, :],
                                         w32[:],
                                         XC[:, 512 * c:512 * c + 512],
                                         tile_position=(0, 32 * r))
                    nc.vector.tensor_scalar(
                        out=dst[:, 512 * g:512 * g + 512], in0=ps[:],
                        scalar1=q2k2b_sb[:, bcol:bcol + 1], scalar2=None,
                        op0=OP.add)

            # ---- v2 direct from XC slices ----
            for jb in range(8):
                vps = ps_o.tile([128, 64], f32, tag="ops", name=f"v2ps{i}{jb}")
                for jj in range(4):
                    j = 4 * jb + jj
                    nc.tensor.matmul(
                        vps[:, 16 * jj:16 * jj + 16],
                        XC[:, 128 * j:128 * j + 128],
                        v2w_sb[:])
                vsrc = vps.rearrange("p (j c) -> p j c", c=16)
                nc.vector.tensor_copy(out=vt23[:, 4 * jb:4 * jb + 4, 0:16], in_=vsrc)

            OC = attp.tile([128, 512], bf16, tag="OC", name=f"OC{i}")
            attention_block(i, q2_sb, k2_sb, vt2, OC, "c")

            # ---- transposes -> XC2 (order-free for ax) ----
            XC2 = attp.tile([16, HW], bf16, tag="XC2", name=f"XC2{i}", bufs=1)
            for b in range(8):
                trp = ps_s.tile([16, 512], bf16, tag="sps", name=f"trc{i}{b}")
                for m in range(4):
                    nc.tensor.transpose(trp[:, 128 * m:128 * m + 128],
                                        OC[:, 64 * b + 16 * m:64 * b + 16 * m + 16],
                                        ident_sb[:])
                nc.vector.tensor_copy(out=XC2[:, 512 * b:512 * b + 512], in_=trp[:])

            # ---- ax projection + tanh + accumulated mean ----
            for ct in range(CT):
                for cp in range(4):
                    ps = ps_big.tile([128, 1024], f32, tag="big",
                                     name=f"axps{i}{ct}{cp}")
                    for half in range(2):
                        c = 2 * cp + half
                        nc.tensor.matmul(ps[:, 512 * half:512 * half + 512],
                                         ax_wT_sb[:, 128 * ct:128 * (ct + 1)],
                                         XC2[:, 512 * c:512 * c + 512])
                    axs = scr.tile([128, 1024], bf16, tag="axs", name=f"axs{i}{ct}{cp}")
                    nc.scalar.activation(out=axs[:], in_=ps[:], func=AF.Tanh,
                                         bias=axbh_sb[ct][:], scale=0.5,
                                         accum_out=ta_cols[i][ct][:, cp:cp + 1])

            # ---- fusion + output (bf16, host upcasts) ----
            for ct in range(CT):
                ta = tiny.tile([128, 1], f32, tag="ta", name=f"tafin{i}{ct}")
                nc.vector.reduce_sum(out=ta[:], in_=ta_cols[i][ct][:], axis=AX)
                f0 = tiny.tile([128, 1], f32, tag="f0", name=f"f0{i}{ct}")
                nc.vector.tensor_scalar(out=f0[:], in0=tg[i][ct][:], scalar1=s_g,
                                        scalar2=K0, op0=OP.mult, op1=OP.add)
                f1 = tiny.tile([128, 1], f32, tag="f1", name=f"f1{i}{ct}")
                nc.vector.scalar_tensor_tensor(out=f1[:], in0=tl[i][ct][:],
                                               scalar=s_l, in1=f0[:],
                                               op0=OP.mult, op1=OP.add)
                fin = tiny.tile([128, 1], f32, tag="fin", name=f"fin{i}{ct}")
                nc.vector.scalar_tensor_tensor(out=fin[:], in0=ta[:],
                                               scalar=s_ax, in1=f1[:],
                                               op0=OP.mult, op1=OP.add)
                outt = outp.tile([128, HW], bf16, tag="outt", name=f"outt{i}{ct}")
                nc.vector.tensor_scalar(out=outt[:], in0=xb[i][ct][:],
                                        scalar1=fin[:], scalar2=None,
                                        op0=OP.mult)
                nc.sync.dma_start(out=dout[i, 128 * ct:128 * (ct + 1), :],
                                  in_=outt[:])

    nc.compile()
    return nc


# ----------------------------------------------------------------------------
# Entry point
# ----------------------------------------------------------------------------
WNAMES = ("dwdiag", "dwb", "dwd1neg", "qrep_wT", "krep_wT", "qkb_rep", "vw",
          "q2w32", "k2w32", "q2k2b_rep", "v2w", "ax_wT", "axb_half", "pw1_wT",
          "pw1b_rep", "sel128", "pw2_wT", "pw2b_half", "fc1_wT", "fc1b",
          "fc2_wT", "fc2b_half", "ident")


def kernel(**inputs):
    from concourse.bass_utils import run_bass_kernel_spmd

    p = host_prep(inputs)
    key = "nc"
    if key not in _cache:
        _cache[key] = build_nc(p)
    nc = _cache[key]

    x = np.asarray(inputs["x"], np.float32).reshape(B, C, HW).astype(BF16)
    wmap = {nm: p[nm] for nm in WNAMES}
    in_maps = [{"x": x[IMGS * c:IMGS * (c + 1)], **wmap} for c in range(NCORES)]
    res = run_bass_kernel_spmd(nc, in_maps, list(range(NCORES)))
    _cache["last_results"] = res
    out = np.concatenate([res.results[c]["out"] for c in range(NCORES)], axis=0)
    return out.astype(np.float32).reshape(B, C, H, W)


# revision 16
# speedup vs baseline: 1.4488x; 1.2918x over previous
"""Trainium2 Bass kernel for nn_EnhancedAttention (sparse axial attention +
SE + local-conv gating, fused output scale). v3.

Sharding: pure data-parallel over batch B=32 across 8 cores (4 images/core);
tiny weights replicated.

v4 changes over v2 (495us baseline):
  - x shipped to device as bf16 (host cast), output returned as bf16 and
    upcast on host: halves both DMA directions, kills on-device casts.
  - XC built in two hops: transposes -> contiguous copies into an h-major
    staging tile (DVE), then the h->w scatter runs on the idle GPSIMD
    engine where it hides under compute.
  - stage-1 q/k/v projections emitted inside Phase A so they fill the PE
    bubbles left by the dw-conv gelu dependency chain.
  - attention block software-pipelined: S(t+1) matmuls issue before
    attn@v(t) so the exp/memset latency is off the PE critical path.
  - pw1 packed 4 column-groups into [128,512] PSUM tiles (tile_position
    col tiling): pw1-gelu runs on 128 lanes instead of 16.
  - S matmuls batched 8 pair-chunks per [128,1024] 2-bank PSUM tile; exp
    is one ACT instr per 8 chunks (4/stage vs 16/stage).
  - final multiply emits bf16 (faster DVE mode).
"""

import numpy as np
import ml_dtypes

B, C, H, W = 32, 256, 64, 64
MID = 16
NCORES = 8
IMGS = B // NCORES  # 4
HW = H * W  # 4096
CT = 2  # channel tiles of 128

_cache = {}

BF16 = ml_dtypes.bfloat16


# ----------------------------------------------------------------------------
# Host-side weight preparation
# ----------------------------------------------------------------------------
def host_prep(inp):
    f32 = np.float32
    p = {}
    row_w = np.asarray(inp["row_w"], f32)   # [48, 256]
    row_b = np.asarray(inp["row_b"], f32)
    col_w = np.asarray(inp["col_w"], f32)   # [48, 16]
    col_b = np.asarray(inp["col_b"], f32)
    ax_w = np.asarray(inp["ax_w"], f32)     # [256, 16]
    ax_b = np.asarray(inp["ax_b"], f32)

    # q/k replicated projections: [ct][128, 128]; block r cols 32r:32r+16
    # hold the weight slice, rest zero (SBUF APs must start at 32-aligned
    # partitions, so q and k live in separate tiles)
    qrep = np.zeros((C, 128), f32)
    krep = np.zeros((C, 128), f32)
    for r in range(4):
        qrep[:, 32 * r:32 * r + 16] = row_w[0:16].T
        krep[:, 32 * r:32 * r + 16] = row_w[16:32].T
    p["qrep_wT"] = qrep.reshape(CT, 128, 128).astype(BF16)
    p["krep_wT"] = krep.reshape(CT, 128, 128).astype(BF16)
    qkb = np.zeros((128, 2), f32)
    for r in range(4):
        qkb[32 * r:32 * r + 16, 0] = row_b[0:16]
        qkb[32 * r:32 * r + 16, 1] = row_b[16:32]
    p["qkb_rep"] = qkb
    row_vb = row_b[32:48]
    # v weights [ct][128, 16]
    p["vw"] = row_w[32:48].T.reshape(CT, 128, 16).astype(BF16)

    # col stage (v bias folded)
    q2w32 = np.zeros((16, 32), f32)
    q2w32[:, 0:16] = col_w[0:16].T
    k2w32 = np.zeros((16, 32), f32)
    k2w32[:, 0:16] = col_w[16:32].T
    p["q2w32"] = q2w32.astype(BF16)
    p["k2w32"] = k2w32.astype(BF16)
    q2k2b = np.zeros((128, 2), f32)
    for r in range(4):
        q2k2b[32 * r:32 * r + 16, 0] = col_b[0:16] + col_w[0:16] @ row_vb
        q2k2b[32 * r:32 * r + 16, 1] = col_b[16:32] + col_w[16:32] @ row_vb
    p["q2k2b_rep"] = q2k2b
    p["v2w"] = col_w[32:48].T.astype(BF16)  # [16, 16]
    col_vb = col_b[32:48] + col_w[32:48] @ row_vb

    p["ax_wT"] = ax_w.T.astype(BF16)  # [16, 256]
    axb = ax_b + ax_w @ col_vb
    p["axb_half"] = (0.5 * axb).reshape(CT, 128, 1).astype(f32)

    # conv branch
    dw1 = np.asarray(inp["dw1_w"], f32)[:, 0, 0, :]  # [256, 3]
    dw2 = np.asarray(inp["dw2_w"], f32)[:, 0, :, 0]  # [256, 3]
    dwd = np.zeros((2, 3, CT, 128, 128), f32)
    for ct in range(CT):
        for tap in range(3):
            dwd[0, tap, ct] = np.diag(dw1[128 * ct:128 * (ct + 1), tap])
            dwd[1, tap, ct] = np.diag(dw2[128 * ct:128 * (ct + 1), tap])
    p["dwdiag"] = dwd.astype(BF16)
    # negated dw1 left/right taps for w-boundary corrections (flat-shift fixup)
    dwn = np.zeros((2, CT, 128, 1), f32)
    for ct in range(CT):
        dwn[0, ct, :, 0] = -dw1[128 * ct:128 * (ct + 1), 0]
        dwn[1, ct, :, 0] = -dw1[128 * ct:128 * (ct + 1), 2]
    p["dwd1neg"] = dwn
    p["dwb"] = np.stack([
        np.asarray(inp["dw1_b"], f32).reshape(CT, 128, 1),
        np.asarray(inp["dw2_b"], f32).reshape(CT, 128, 1),
    ])  # [2, CT, 128, 1]
    # pw1: packed 4 column-groups; lhsT [ct][128, 32] (cols 16:32 zero so the
    # matmul writes zeros to the unused partition rows of each 32-group)
    pw1pad = np.zeros((CT, 128, 32), f32)
    pw1w = np.asarray(inp["pw1_w"], f32)[:, :, 0, 0]  # [16, 256]
    for ct in range(CT):
        pw1pad[ct, :, 0:16] = pw1w[:, 128 * ct:128 * (ct + 1)].T
    p["pw1_wT"] = pw1pad.astype(BF16)
    pw1b_rep = np.zeros((128, 1), f32)
    for g in range(4):
        pw1b_rep[32 * g:32 * g + 16, 0] = np.asarray(inp["pw1_b"], f32)
    p["pw1b_rep"] = pw1b_rep
    # selection matrix: out[c] = sum_g lsum128[32g+c]
    sel = np.zeros((128, 16), f32)
    for g in range(4):
        for j in range(16):
            sel[32 * g + j, j] = 1.0
    p["sel128"] = sel
    p["pw2_wT"] = (np.asarray(inp["pw2_w"], f32)[:, :, 0, 0] / HW).T.copy()  # [16, 256] f32
    p["pw2b_half"] = (0.5 * np.asarray(inp["pw2_b"], f32)).reshape(CT, 128, 1).copy()

    # SE (fp32 throughout, tiny)
    p["fc1_wT"] = (np.asarray(inp["fc1_w"], f32) / HW).T.reshape(CT, 128, 16).copy()
    p["fc1b"] = np.asarray(inp["fc1_b"], f32).reshape(16, 1)
    p["fc2_wT"] = np.asarray(inp["fc2_w"], f32).T.copy()  # [16, 256]
    p["fc2b_half"] = (0.5 * np.asarray(inp["fc2_b"], f32)).reshape(CT, 128, 1).copy()

    p["ident"] = np.eye(128, dtype=f32).astype(BF16)

    fwin = np.asarray(inp["fusion_w"], np.float64)
    e = np.exp(fwin - fwin.max())
    fw = e / e.sum()
    p["_K0"] = float(0.5 * (fw[0] + fw[1] + fw[2]) + fw[3])
    p["_s_g"] = float(0.5 * fw[0])
    p["_s_l"] = float(0.5 * fw[1])
    p["_s_ax"] = float(0.5 * fw[2] / HW)
    return p


# ----------------------------------------------------------------------------
# Bass kernel construction
# ----------------------------------------------------------------------------
def build_nc(scalars, n_imgs=IMGS):
    import concourse.bacc as bacc
    import concourse.bass as bass
    import concourse.tile as tile
    from concourse import mybir

    f32 = mybir.dt.float32
    bf16 = mybir.dt.bfloat16
    AX = mybir.AxisListType.X
    OP = mybir.AluOpType
    AF = mybir.ActivationFunctionType

    nc = bacc.Bacc("TRN2", target_bir_lowering=False, debug=False,
                   num_devices=NCORES)

    # ---- DRAM tensors ----
    dx = nc.dram_tensor("x", [n_imgs, C, HW], bf16, kind="ExternalInput")
    dout = nc.dram_tensor("out", [n_imgs, C, HW], bf16, kind="ExternalOutput")
    dw_names = [
        ("dwdiag", [2, 3, CT, 128, 128], bf16), ("dwb", [2, CT, 128, 1], f32),
        ("dwd1neg", [2, CT, 128, 1], f32),
        ("qrep_wT", [CT, 128, 128], bf16), ("krep_wT", [CT, 128, 128], bf16),
        ("qkb_rep", [128, 2], f32),
        ("vw", [CT, 128, 16], bf16),
        ("q2w32", [16, 32], bf16), ("k2w32", [16, 32], bf16),
        ("q2k2b_rep", [128, 2], f32),
        ("v2w", [16, 16], bf16),
        ("ax_wT", [16, 256], bf16), ("axb_half", [CT, 128, 1], f32),
        ("pw1_wT", [CT, 128, 32], bf16), ("pw1b_rep", [128, 1], f32),
        ("sel128", [128, 16], f32),
        ("pw2_wT", [16, 256], f32), ("pw2b_half", [CT, 128, 1], f32),
        ("fc1_wT", [CT, 128, 16], f32), ("fc1b", [16, 1], f32),
        ("fc2_wT", [16, 256], f32), ("fc2b_half", [CT, 128, 1], f32),
        ("ident", [128, 128], bf16),
    ]
    dws = {nm: nc.dram_tensor(nm, sh, dt, kind="ExternalInput")
           for nm, sh, dt in dw_names}

    K0, s_g, s_l, s_ax = (scalars["_K0"], scalars["_s_g"],
                          scalars["_s_l"], scalars["_s_ax"])

    from contextlib import ExitStack
    with tile.TileContext(nc) as tc, ExitStack() as es:
        singles = es.enter_context(tc.tile_pool(name="singles", bufs=1))
        xbp = es.enter_context(tc.tile_pool(name="xbp", bufs=1))
        y1p = es.enter_context(tc.tile_pool(name="y1p", bufs=1))
        scr = es.enter_context(tc.tile_pool(name="scr", bufs=2))
        attp = es.enter_context(tc.tile_pool(name="attp", bufs=2))
        outp = es.enter_context(tc.tile_pool(name="outp", bufs=1))
        tiny = es.enter_context(tc.tile_pool(name="tiny", bufs=8))
        gate = es.enter_context(tc.tile_pool(name="gate", bufs=1))
        # PSUM: big [128,1024] 2-bank x2 = 4; sps [128,512] 1-bank x2 = 2;
        # ops 1-bank x2 = 2  => 8 banks
        ps_big = es.enter_context(tc.tile_pool(name="ps_big", bufs=2, space="PSUM"))
        ps_s = es.enter_context(tc.tile_pool(name="ps_s", bufs=2, space="PSUM"))
        ps_o = es.enter_context(tc.tile_pool(name="ps_o", bufs=2, space="PSUM"))

        # ---- load weights to SBUF (scalar HWDGE queue so the x-input DMAs
        # on the sync queue start immediately) ----
        def wtile(name, shape, dt, src):
            t = singles.tile(shape, dt, tag=name, name=name)
            nc.scalar.dma_start(out=t[:], in_=src)
            return t

        dwd_sb = [[[wtile(f"dwd{st}{tap}{ct}", [128, 128], bf16,
                          dws["dwdiag"][st, tap, ct])
                    for ct in range(CT)] for tap in range(3)] for st in range(2)]
        dwb_sb = [[wtile(f"dwb{st}{ct}", [128, 1], f32, dws["dwb"][st, ct])
                   for ct in range(CT)] for st in range(2)]
        dwn_sb = [[wtile(f"dwn{sd}{ct}", [128, 1], f32, dws["dwd1neg"][sd, ct])
                   for ct in range(CT)] for sd in range(2)]
        qrep_sb = [wtile(f"qrep{ct}", [128, 128], bf16, dws["qrep_wT"][ct])
                   for ct in range(CT)]
        krep_sb = [wtile(f"krep{ct}", [128, 128], bf16, dws["krep_wT"][ct])
                   for ct in range(CT)]
        qkb_sb = wtile("qkb", [128, 2], f32, dws["qkb_rep"][:])
        vw_sb = [wtile(f"vw{ct}", [128, 16], bf16, dws["vw"][ct]) for ct in range(CT)]
        q2w32_sb = wtile("q2w32", [16, 32], bf16, dws["q2w32"][:])
        k2w32_sb = wtile("k2w32", [16, 32], bf16, dws["k2w32"][:])
        q2k2b_sb = wtile("q2k2b", [128, 2], f32, dws["q2k2b_rep"][:])
        v2w_sb = wtile("v2w", [16, 16], bf16, dws["v2w"][:])
        ax_wT_sb = wtile("axwT", [16, 256], bf16, dws["ax_wT"][:])
        axbh_sb = [wtile(f"axbh{ct}", [128, 1], f32, dws["axb_half"][ct])
                   for ct in range(CT)]
        pw1_sb = [wtile(f"pw1{ct}", [128, 32], bf16, dws["pw1_wT"][ct])
                  for ct in range(CT)]
        pw1br_sb = wtile("pw1br", [128, 1], f32, dws["pw1b_rep"][:])
        sel_sb = wtile("sel128", [128, 16], f32, dws["sel128"][:])
        pw2_sb = wtile("pw2", [16, 256], f32, dws["pw2_wT"][:])
        pw2bh_sb = [wtile(f"pw2bh{ct}", [128, 1], f32, dws["pw2b_half"][ct])
                    for ct in range(CT)]
        fc1_sb = [wtile(f"fc1{ct}", [128, 16], f32, dws["fc1_wT"][ct])
                  for ct in range(CT)]
        fc1b_sb = wtile("fc1b", [16, 1], f32, dws["fc1b"][:])
        fc2_sb = wtile("fc2", [16, 256], f32, dws["fc2_wT"][:])
        fc2bh_sb = [wtile(f"fc2bh{ct}", [128, 1], f32, dws["fc2b_half"][ct])
                    for ct in range(CT)]
        ident_sb = wtile("ident", [128, 128], bf16, dws["ident"][:])

        # persistent vt (per image, filled during Phase A) / vt2 ([px, 17]
        # with ones col); ones cols written once
        vt = []
        vt3 = []
        for i in range(n_imgs):
            v = singles.tile([128, 544], bf16, tag=f"vtP{i}", name=f"vtP{i}")
            v3 = v.rearrange("p (j c) -> p j c", c=17)
            nc.vector.memset(v3[:, :, 16], 1.0)
            vt.append(v)
            vt3.append(v3)
        vt2 = singles.tile([128, 544], bf16, tag="vt2P", name="vt2P")
        vt23 = vt2.rearrange("p (j c) -> p j c", c=17)
        nc.vector.memset(vt23[:, :, 16], 1.0)

        # persistent expS tiles [128, 1024] (8 pair-chunks each), fully zeroed
        # once; exp writes the whole tile, then the 64x64 cross blocks are
        # re-zeroed (2 strided DVE memsets) so attn@v runs as one K=128
        # matmul per pair
        expS_t = {}
        for pfx in ("r", "c"):
            for par in range(2):
                e = singles.tile([128, 1024], bf16, tag=f"expS{pfx}{par}",
                                 name=f"expS{pfx}{par}")
                nc.vector.memset(e[:], 0.0)
                expS_t[(pfx, par)] = e

        # persistent bf16 x for all images (DMA'd directly, host pre-cast)
        xb = [[xbp.tile([128, HW], bf16, tag=f"xb{i}_{ct}", name=f"xb{i}_{ct}")
               for ct in range(CT)] for i in range(n_imgs)]
        # gates per image
        tg = [[gate.tile([128, 1], f32, tag=f"tg{i}{ct}", name=f"tg{i}{ct}")
               for ct in range(CT)] for i in range(n_imgs)]
        tl = [[gate.tile([128, 1], f32, tag=f"tl{i}{ct}", name=f"tl{i}{ct}")
               for ct in range(CT)] for i in range(n_imgs)]
        ta_cols = [[gate.tile([128, 4], f32, tag=f"ta{i}{ct}", name=f"ta{i}{ct}")
                    for ct in range(CT)] for i in range(n_imgs)]

        # ==================== Phase A: conv + SE (gelu table) ================
        qk_sbs = []
        for i in range(n_imgs):
            for ct in range(CT):
                nc.sync.dma_start(out=xb[i][ct][:],
                                  in_=dx[i, 128 * ct:128 * (ct + 1), :])

            # ---- SE gate ----
            gsum = [tiny.tile([128, 1], f32, tag="gsum", name=f"gsum{i}{ct}")
                    for ct in range(CT)]
            for ct in range(CT):
                nc.vector.reduce_sum(out=gsum[ct][:], in_=xb[i][ct][:], axis=AX)
            fc1ps = ps_o.tile([16, 1], f32, tag="ops", name=f"fc1ps{i}")
            for ct in range(CT):
                nc.tensor.matmul(fc1ps[:], fc1_sb[ct][:], gsum[ct][:],
                                 start=(ct == 0), stop=(ct == 1))
            r1 = tiny.tile([16, 1], f32, tag="r1", name=f"r1{i}")
            nc.scalar.activation(out=r1[:], in_=fc1ps[:], func=AF.Relu,
                                 bias=fc1b_sb[:], scale=1.0)
            for ct in range(CT):
                fc2ps = ps_o.tile([128, 1], f32, tag="ops", name=f"fc2ps{i}{ct}")
                nc.tensor.matmul(fc2ps[:], fc2_sb[:, 128 * ct:128 * (ct + 1)], r1[:])
                nc.scalar.activation(out=tg[i][ct][:], in_=fc2ps[:], func=AF.Tanh,
                                     bias=fc2bh_sb[ct][:], scale=0.5)

            # ---- dw1 (1x3 along w): 2-chunk PSUM pairs ----
            y1 = [y1p.tile([128, HW], bf16, tag=f"y1{ct}", name=f"y1{i}{ct}")
                  for ct in range(CT)]
            for ct in range(CT):
                xb3 = xb[i][ct].rearrange("p (h w) -> p h w", w=64)
                for cp in range(4):  # chunk pairs
                    ps = ps_big.tile([128, 1024], f32, tag="big", name=f"dw1ps{i}{ct}{cp}")
                    ps3 = ps.rearrange("p (h w) -> p h w", w=64)  # 16 h-rows
                    for half in range(2):
                        c = 2 * cp + half
                        o = 512 * c
                        po = 512 * half
                        nc.tensor.matmul(ps[:, po:po + 512], dwd_sb[0][1][ct][:],
                                         xb[i][ct][:, o:o + 512],
                                         start=True, stop=False)
                        lo = 1 if c == 0 else 0
                        nc.tensor.matmul(ps[:, po + lo:po + 512], dwd_sb[0][0][ct][:],
                                         xb[i][ct][:, o + lo - 1:o + 511],
                                         start=False, stop=False)
                        hi = 511 if c == 7 else 512
                        nc.tensor.matmul(ps[:, po:po + hi], dwd_sb[0][2][ct][:],
                                         xb[i][ct][:, o + 1:o + 1 + hi],
                                         start=False, stop=True)
                    # subtract wrapped left tap at w=0 (h>0), right tap at
                    # w=63 (h<63) -- both 1024-col halves in one STT each
                    lh = 1 if cp == 0 else 0
                    nc.vector.scalar_tensor_tensor(
                        out=ps3[:, lh:16, 0],
                        in0=xb3[:, 16 * cp + lh - 1:16 * cp + 15, 63],
                        scalar=dwn_sb[0][ct][:], in1=ps3[:, lh:16, 0],
                        op0=OP.mult, op1=OP.add)
                    rh = 15 if cp == 3 else 16
                    nc.vector.scalar_tensor_tensor(
                        out=ps3[:, 0:rh, 63],
                        in0=xb3[:, 16 * cp + 1:16 * cp + 1 + rh, 0],
                        scalar=dwn_sb[1][ct][:], in1=ps3[:, 0:rh, 63],
                        op0=OP.mult, op1=OP.add)
                    nc.scalar.activation(out=y1[ct][:, 1024 * cp:1024 * (cp + 1)],
                                         in_=ps[:], func=AF.Gelu,
                                         bias=dwb_sb[0][ct][:], scale=1.0)

            # ---- stage-1 q/k/v projections (emitted here so they fill the
            # PE bubbles left by the dw gelu dependency chain) ----
            # q/k: q_sb/k_sb [128, 1024]; block r rows 32r:32r+16 = chunk
            # 4g+r at cols 512g
            q_sb = attp.tile([128, 1024], bf16, tag="qt", name=f"q{i}", bufs=4)
            k_sb = attp.tile([128, 1024], bf16, tag="kt", name=f"k{i}", bufs=4)
            qk_sbs.append((q_sb, k_sb))
            for g in range(2):
                for rep, dst, bcol in ((qrep_sb, q_sb, 0), (krep_sb, k_sb, 1)):
                    ps = ps_s.tile([128, 512], f32, tag="sps",
                                   name=f"qkps{i}{g}{bcol}")
                    for r in range(4):
                        c = 4 * g + r
                        for ct in range(CT):
                            nc.tensor.matmul(
                                ps[32 * r:32 * r + 32, :],
                                rep[ct][:, 32 * r:32 * r + 32],
                                xb[i][ct][:, 512 * c:512 * c + 512],
                                start=(ct == 0), stop=(ct == 1),
                                tile_position=(0, 32 * r))
                    nc.vector.tensor_scalar(
                        out=dst[:, 512 * g:512 * g + 512], in0=ps[:],
                        scalar1=qkb_sb[:, bcol:bcol + 1], scalar2=None,
                        op0=OP.add)
            # v direct: vt[i] [128, 544] = [px-pair, 17] with ones col
            for jb in range(8):
                vps = ps_o.tile([128, 64], f32, tag="ops", name=f"vps{i}{jb}")
                for jj in range(4):
                    j = 4 * jb + jj
                    for ct in range(CT):
                        nc.tensor.matmul(
                            vps[:, 16 * jj:16 * jj + 16],
                            xb[i][ct][:, 128 * j:128 * j + 128],
                            vw_sb[ct][:],
                            start=(ct == 0), stop=(ct == 1))
                vsrc = vps.rearrange("p (j c) -> p j c", c=16)
                nc.vector.tensor_copy(out=vt3[i][:, 4 * jb:4 * jb + 4, 0:16],
                                      in_=vsrc)

            # ---- dw2 (3x1 along h) -> gelu -> pw1 (4 col-groups packed) ----
            lsum_cols = tiny.tile([128, 2], f32, tag="lsum_cols", name=f"lsc{i}")
            for cp in range(4):
                y2c = []
                for ct in range(CT):
                    ps = ps_big.tile([128, 1024], f32, tag="big",
                                     name=f"dw2ps{i}{ct}{cp}")
                    for half in range(2):
                        c = 2 * cp + half
                        o = 512 * c
                        po = 512 * half
                        nc.tensor.matmul(ps[:, po:po + 512], dwd_sb[1][1][ct][:],
                                         y1[ct][:, o:o + 512],
                                         start=True, stop=False)
                        if c == 0:
                            nc.tensor.matmul(ps[:, po + 64:po + 512],
                                             dwd_sb[1][0][ct][:],
                                             y1[ct][:, 0:448],
                                             start=False, stop=False)
                        else:
                            nc.tensor.matmul(ps[:, po:po + 512], dwd_sb[1][0][ct][:],
                                             y1[ct][:, o - 64:o + 448],
                                             start=False, stop=False)
                        if c == 7:
                            nc.tensor.matmul(ps[:, po:po + 448], dwd_sb[1][2][ct][:],
                                             y1[ct][:, o + 64:o + 512],
                                             start=False, stop=True)
                        else:
                            nc.tensor.matmul(ps[:, po:po + 512], dwd_sb[1][2][ct][:],
                                             y1[ct][:, o + 64:o + 576],
                                             start=False, stop=True)
                    yc = scr.tile([128, 1024], bf16, tag=f"y2c{ct}",
                                  name=f"y2c{i}{ct}{cp}")
                    nc.scalar.activation(out=yc[:], in_=ps[:], func=AF.Gelu,
                                         bias=dwb_sb[1][ct][:], scale=1.0)
                    y2c.append(yc)
                # pw1: half-chunk c -> col group g=c%4, accumulated over ct.
                # Two [128,512] PSUM tiles per image (cp pairs 0-1 and 2-3).
                if cp % 2 == 0:
                    pw1ps = ps_s.tile([128, 512], f32, tag="sps",
                                      name=f"pw1ps{i}{cp // 2}")
                for half in range(2):
                    g = (2 * cp + half) % 4
                    for ct in range(CT):
                        nc.tensor.matmul(pw1ps[32 * g:32 * g + 32, :],
                                         pw1_sb[ct][:],
                                         y2c[ct][:, 512 * half:512 * half + 512],
                                         start=(ct == 0), stop=(ct == 1),
                                         tile_position=(0, 32 * g))
                if cp % 2 == 1:
                    g3 = scr.tile([128, 512], bf16, tag="g3", name=f"g3{i}{cp // 2}")
                    nc.scalar.activation(out=g3[:], in_=pw1ps[:], func=AF.Gelu,
                                         bias=pw1br_sb[:], scale=1.0,
                                         accum_out=lsum_cols[:, cp // 2:cp // 2 + 1])

            # local gate: fold the 4 col-groups with a tiny matmul, then pw2
            lsum_ps = ps_o.tile([16, 2], f32, tag="ops", name=f"lsps{i}")
            nc.tensor.matmul(lsum_ps[:], sel_sb[:], lsum_cols[:])
            lsum2 = tiny.tile([16, 2], f32, tag="lsum2", name=f"lsum2{i}")
            nc.vector.tensor_copy(out=lsum2[:], in_=lsum_ps[:])
            lsum = tiny.tile([16, 1], f32, tag="lsum", name=f"lsum{i}")
            nc.vector.reduce_sum(out=lsum[:], in_=lsum2[:], axis=AX)
            for ct in range(CT):
                ps = ps_o.tile([128, 1], f32, tag="ops", name=f"pw2ps{i}{ct}")
                nc.tensor.matmul(ps[:], pw2_sb[:, 128 * ct:128 * (ct + 1)], lsum[:])
                nc.scalar.activation(out=tl[i][ct][:], in_=ps[:], func=AF.Tanh,
                                     bias=pw2bh_sb[ct][:], scale=0.5)

        # ==================== Phase B: axial attention (exp table) ===========
        def attention_block(i, qt, kt, vtt, OC_dst, pfx):
            """S^T matmuls (8 pair-chunks per [128,1024] PSUM tile) -> one exp
            -> re-zero cross blocks -> attn@v + denom -> normalize.
            Software-pipelined: S(t+1) issues before attn@v(t) so the
            exp/memset chain stays off the PE critical path."""
            vt3l = vtt.rearrange("p (j c) -> p j c", c=17)

            def stage_S(t):
                Sps = ps_big.tile([128, 1024], f32, tag="big",
                                  name=f"S{pfx}{i}{t}")
                for s in range(8):
                    j = 8 * t + s
                    cch = j // 4
                    r, g = cch % 4, cch // 4
                    sl = slice(32 * r, 32 * r + 16)
                    fo = 512 * g + 128 * (j % 4)
                    nc.tensor.matmul(
                        Sps[:, 128 * s:128 * s + 128],
                        kt[sl, fo:fo + 128], qt[sl, fo:fo + 128],
                        tile_position=(32 * r, 0))
                expS = expS_t[(pfx, t % 2)]
                nc.scalar.activation(out=expS[:], in_=Sps[:], func=AF.Exp,
                                     scale=0.25)
                e4 = expS.rearrange("p (u dh c) -> p u dh c", dh=2, c=64)
                nc.vector.memset(e4[0:64, :, 1], 0.0)
                nc.vector.memset(e4[64:128, :, 0], 0.0)

            def stage_AV(t):
                expS = expS_t[(pfx, t % 2)]
                Ops = ps_o.tile([128, 136], f32, tag="ops", name=f"O{pfx}{i}{t}")
                for s in range(8):
                    j = 8 * t + s
                    nc.tensor.matmul(
                        Ops[:, 17 * s:17 * s + 17],
                        expS[:, 128 * s:128 * s + 128],
                        vt3l[:, j, :])
                O3 = Ops.rearrange("p (s c) -> p s c", c=17)
                rD = tiny.tile([128, 8], f32, tag="rD", name=f"rD{pfx}{i}{t}")
                nc.vector.reciprocal(out=rD[:], in_=O3[:, :, 16])
                rDb = bass.AP(tensor=rD.tensor, offset=rD.offset,
                              ap=[rD.ap[0], [1, 8], [0, 16]])
                dst3 = OC_dst[:, 128 * t:128 * t + 128].rearrange(
                    "p (s c) -> p s c", c=16)
                nc.vector.tensor_tensor(out=dst3[:], in0=O3[:, :, 0:16],
                                        in1=rDb, op=OP.mult)

            stage_S(0)
            for t in range(1, 4):
                stage_S(t)
                stage_AV(t - 1)
            stage_AV(3)

        # ---- stage 1: row attention -> XC, per image ----
        XCs = {}

        def stage1(i):
            q_sb, k_sb = qk_sbs[i]
            OR = attp.tile([128, 512], bf16, tag="OR", name=f"OR{i}")
            attention_block(i, q_sb, k_sb, vt[i], OR, "r")

            # ---- transposes -> XCh [16, (h w)] staging (contiguous DVE
            # copies), then h->w scatter to XC [16, (w h)] on idle GPSIMD ----
            XCh = attp.tile([16, HW], bf16, tag="XChst", name=f"XCh{i}", bufs=2)
            for b in range(8):
                trp = ps_s.tile([16, 512], bf16, tag="sps", name=f"trp{i}{b}")
                for m in range(4):
                    nc.tensor.transpose(trp[:, 128 * m:128 * m + 128],
                                        OR[:, 64 * b + 16 * m:64 * b + 16 * m + 16],
                                        ident_sb[:])
                nc.vector.tensor_copy(out=XCh[:, 512 * b:512 * b + 512], in_=trp[:])
            XC = attp.tile([16, HW], bf16, tag="XCimg", name=f"XC{i}", bufs=2)
            XCh3 = XCh.rearrange("d (h w) -> d w h", w=64)
            for b in range(8):
                nc.gpsimd.tensor_copy(out=XC[:, 512 * b:512 * b + 512],
                                      in_=XCh3[:, 8 * b:8 * b + 8, :])
            XCs[i] = XC

        # ---- stage 2: col attention -> ax -> fusion + output, per image ----
        def stage2(i):
            XC = XCs[i]
            # ---- col stage: q2/k2 projections ----
            q2_sb = attp.tile([128, 1024], bf16, tag="q2t", name=f"q2{i}")
            k2_sb = attp.tile([128, 1024], bf16, tag="k2t", name=f"k2{i}")
            for g in range(2):
                for w32, dst, bcol in ((q2w32_sb, q2_sb, 0), (k2w32_sb, k2_sb, 1)):
                    ps = ps_s.tile([128, 512], f32, tag="sps",
                                   name=f"qk2ps{i}{g}{bcol}")
                    for r in range(4):
                        c = 4 * g + r
                        nc.tensor.matmul(ps[32 * r:32 * r + 32, :],
                                         w32[:],
                                         XC[:, 512 * c:512 * c + 512],
                                         tile_position=(0, 32 * r))
                    nc.vector.tensor_scalar(
                        out=dst[:, 512 * g:512 * g + 512], in0=ps[:],
                        scalar1=q2k2b_sb[:, bcol:bcol + 1], scalar2=None,
                        op0=OP.add)

            # ---- v2 direct from XC slices ----
            for jb in range(8):
                vps = ps_o.tile([128, 64], f32, tag="ops", name=f"v2ps{i}{jb}")
                for jj in range(4):
                    j = 4 * jb + jj
                    nc.tensor.matmul(
                        vps[:, 16 * jj:16 * jj + 16],
                        XC[:, 128 * j:128 * j + 128],
                        v2w_sb[:])
                vsrc = vps.rearrange("p (j c) -> p j c", c=16)
                nc.vector.tensor_copy(out=vt23[:, 4 * jb:4 * jb + 4, 0:16], in_=vsrc)

            OC = attp.tile([128, 512], bf16, tag="OC", name=f"OC{i}")
            attention_block(i, q2_sb, k2_sb, vt2, OC, "c")

            # ---- transposes -> XC2 (order-free for ax) ----
            XC2 = attp.tile([16, HW], bf16, tag="XC2", name=f"XC2{i}", bufs=1)
            for b in range(8):
                trp = ps_s.tile([16, 512], bf16, tag="sps", name=f"trc{i}{b}")
                for m in range(4):
                    nc.tensor.transpose(trp[:, 128 * m:128 * m + 128],
                                        OC[:, 64 * b + 16 * m:64 * b + 16 * m + 16],
                                        ident_sb[:])
                nc.vector.tensor_copy(out=XC2[:, 512 * b:512 * b + 512], in_=trp[:])

            # ---- ax projection + tanh + accumulated mean ----
            for ct in range(CT):
                for cp in range(4):
                    ps = ps_big.tile([128, 1024], f32, tag="big",
                                     name=f"axps{i}{ct}{cp}")
                    for half in range(2):
                        c = 2 * cp + half
                        nc.tensor.matmul(ps[:, 512 * half:512 * half + 512],
                                         ax_wT_sb[:, 128 * ct:128 * (ct + 1)],
                                         XC2[:, 512 * c:512 * c + 512])
                    axs = scr.tile([128, 1024], bf16, tag="axs", name=f"axs{i}{ct}{cp}")
                    nc.scalar.activation(out=axs[:], in_=ps[:], func=AF.Tanh,
                                         bias=axbh_sb[ct][:], scale=0.5,
                                         accum_out=ta_cols[i][ct][:, cp:cp + 1])

            # ---- fusion + output (bf16, host upcasts) ----
            for ct in range(CT):
                ta = tiny.tile([128, 1], f32, tag="ta", name=f"tafin{i}{ct}")
                nc.vector.reduce_sum(out=ta[:], in_=ta_cols[i][ct][:], axis=AX)
                f0 = tiny.tile([128, 1], f32, tag="f0", name=f"f0{i}{ct}")
                nc.vector.tensor_scalar(out=f0[:], in0=tg[i][ct][:], scalar1=s_g,
                                        scalar2=K0, op0=OP.mult, op1=OP.add)
                f1 = tiny.tile([128, 1], f32, tag="f1", name=f"f1{i}{ct}")
                nc.vector.scalar_tensor_tensor(out=f1[:], in0=tl[i][ct][:],
                                               scalar=s_l, in1=f0[:],
                                               op0=OP.mult, op1=OP.add)
                fin = tiny.tile([128, 1], f32, tag="fin", name=f"fin{i}{ct}")
                nc.vector.scalar_tensor_tensor(out=fin[:], in0=ta[:],
                                               scalar=s_ax, in1=f1[:],
                                               op0=OP.mult, op1=OP.add)
                outt = outp.tile([128, HW], bf16, tag="outt", name=f"outt{i}{ct}")
                nc.vector.tensor_scalar(out=outt[:], in0=xb[i][ct][:],
                                        scalar1=fin[:], scalar2=None,
                                        op0=OP.mult)
                nc.sync.dma_start(out=dout[i, 128 * ct:128 * (ct + 1), :],
                                  in_=outt[:])

        # interleave so GPSIMD scatters and stage-2 ACT work overlap the
        # next image's stage-1 PE work
        stage1(0)
        stage1(1)
        stage2(0)
        stage1(2)
        stage2(1)
        stage1(3)
        stage2(2)
        stage2(3)

    nc.compile()
    return nc


# ----------------------------------------------------------------------------
# Entry point
# ----------------------------------------------------------------------------
WNAMES = ("dwdiag", "dwb", "dwd1neg", "qrep_wT", "krep_wT", "qkb_rep", "vw",
          "q2w32", "k2w32", "q2k2b_rep", "v2w", "ax_wT", "axb_half", "pw1_wT",
          "pw1b_rep", "sel128", "pw2_wT", "pw2b_half", "fc1_wT", "fc1b",
          "fc2_wT", "fc2b_half", "ident")


def kernel(**inputs):
    from concourse.bass_utils import run_bass_kernel_spmd

    p = host_prep(inputs)
    key = "nc"
    if key not in _cache:
        _cache[key] = build_nc(p)
    nc = _cache[key]

    x = np.asarray(inputs["x"], np.float32).reshape(B, C, HW).astype(BF16)
    wmap = {nm: p[nm] for nm in WNAMES}
    in_maps = [{"x": x[IMGS * c:IMGS * (c + 1)], **wmap} for c in range(NCORES)]
    res = run_bass_kernel_spmd(nc, in_maps, list(range(NCORES)))
    _cache["last_results"] = res
    out = np.concatenate([res.results[c]["out"] for c in range(NCORES)], axis=0)
    return out.astype(np.float32).reshape(B, C, H, W)
